# revision 1
# baseline (speedup 1.0000x reference)
"""Trainium2 Bass kernel for nn_AttentionHead (B=4, N=2048, d_model=1024, d_k=64).

Sharding: data-parallel over (batch, query-half) -> 8 cores. Each core gets
q^T[b, :, h*1024:(h+1)*1024], full k^T[b], v^T[b] (host pre-transposes so
d_model lands on SBUF partitions; projections contract d_model), plus the
packed projection weights. All matmuls are bf16 with fp32 PSUM accumulation.

Per-core device program (single pass, engines overlapped):
  1. k/q projections ride a dual-queue (SP+ACT HWDGE) DMA stream of kT/qT
     granules; per-chunk psum accumulators write back k_^T[64,2048] and
     q_^T[64,1024] (zero-padded to 128 partitions), biases folded in.
  2. t-major attention loop over 16 key tiles: scores^T tile
     [128,1024] = k-slice^T q_^T in PSUM, one Exp ACTIVATE per tile
     (scale=1/sqrt(dk) folded in) into a persistent e buffer. The v chain
     (chunk-major vT DMA granules -> projection -> bf16 PE transpose into
     v_aug[nk,65] with a ones column) and the out-matmul accumulation
     out_aug^T[65,512] += v_aug_t^T e_t are hand-interleaved into the PE
     instruction stream. The ones column makes row 64 the softmax
     denominator (unstabilized exp-softmax, faithful to the reference).
  3. Normalize in transposed layout: reciprocal of the denominator row,
     broadcast across partitions with a K=1 matmul, fp32 multiply; the
     [64, 1024] result is DMA'd out and de-transposed on the host.

A small legalization pass hoists excess per-instruction semaphore waits
onto same-engine NoOps (this container's walrus accepts at most one).
"""

import numpy as np
import ml_dtypes

import concourse.bass as bass
import concourse.tile as tile
from concourse import mybir
from concourse.bass_utils import run_bass_kernel_spmd
from concourse.masks import make_identity

B, N, DM, DK = 4, 2048, 1024, 64
NCORES = 8
NQ = N // 2          # queries per core
NK = N               # keys per core
P = 128
NDM = DM // P        # 8 d_model tiles
NKT = NK // P        # 16 key tiles
NQC = 512            # query chunk (one PSUM bank)
NQCH = NQ // NQC     # 2 query chunks
DT = mybir.dt.bfloat16
F32 = mybir.dt.float32
BF = ml_dtypes.bfloat16


# --- walrus wait legalization -------------------------------------------------
# The walrus build in this container accepts at most 1 sync wait + 1 sync
# update per instruction (2 for EventSemaphore). Excess WAITS are hoisted
# onto same-engine NoOps placed just before (queues issue in order, so the
# gating is preserved). Updates are completion-signals and stay put.

def _caps(inst):
    opcode = type(inst).__name__
    if opcode == "InstEventSemaphore":
        return 2, 2
    return 1, 1


def _legalize_waits(nc):
    for f in nc.m.functions:
        for bb in f.blocks:
            out = []
            changed = False
            for inst in bb.instructions:
                si = inst.sync_info
                waits = list(si.on_wait) if si is not None else []
                updates = list(si.on_update) if si is not None else []
                wcap, ucap = _caps(inst)
                if len(waits) <= wcap and len(updates) <= ucap:
                    out.append(inst)
                    continue
                changed = True
                keep_w = waits[len(waits) - wcap:] if wcap else []
                extra_w = waits[: len(waits) - wcap] if wcap else waits
                # Updates signal instruction COMPLETION (writes landed);
                # a following NoOp fires at issue time instead, which races
                # consumers against in-flight writes. Never hoist them.
                assert len(updates) <= ucap, (
                    f"{inst.name}: {len(updates)} sync updates exceed the "
                    f"per-instruction cap and cannot be hoisted safely"
                )
                keep_u = updates
                extra_u = []
                for w in extra_w:
                    nop = mybir.InstNoOp(
                        name=nc.get_next_instruction_name(), ins=[], outs=[]
                    )
                    nop.engine = inst.engine
                    nop.sync_info = mybir.SyncInfo(on_wait=[w], on_update=[])
                    out.append(nop)
                inst.sync_info = mybir.SyncInfo(on_wait=keep_w, on_update=keep_u)
                out.append(inst)
                for u in extra_u:
                    nop = mybir.InstNoOp(
                        name=nc.get_next_instruction_name(), ins=[], outs=[]
                    )
                    nop.engine = inst.engine
                    nop.sync_info = mybir.SyncInfo(on_wait=[], on_update=[u])
                    out.append(nop)
            if changed:
                bb.instructions = out


# --- device program -----------------------------------------------------------

def _build(reps=1):
    nc = bass.Bass()
    qT_d = nc.dram_tensor("qT", [DM, NQ], DT, kind="ExternalInput")
    kT_d = nc.dram_tensor("kT", [DM, NK], DT, kind="ExternalInput")
    vT_d = nc.dram_tensor("vT", [DM, NK], DT, kind="ExternalInput")
    w3_d = nc.dram_tensor("w3", [P, NDM * 3 * DK], DT, kind="ExternalInput")
    b3_d = nc.dram_tensor("b3", [DK, 3], F32, kind="ExternalInput")
    out_d = nc.dram_tensor("out", [DK, NQ], F32, kind="ExternalOutput")

    NCH_K = NK // NQC   # 4 key chunks
    NCH_Q = NQ // NQC   # 2 query chunks
    EXP = mybir.ActivationFunctionType.Exp
    IDF = mybir.ActivationFunctionType.Identity
    SCALE = 1.0 / float(np.sqrt(np.float32(DK)))

    with tile.TileContext(nc) as tc:
      for _rep in range(reps):
        with tc.tile_pool(name="persist", bufs=1) as persist:
            w3_sb = persist.tile([P, NDM, 3 * DK], DT, tag="w3_sb")
            b3_sb = persist.tile([DK, 3], F32, tag="b3_sb")
            ident = persist.tile([P, P], F32, tag="ident")
            identb = persist.tile([P, P], DT, tag="identb")
            k_sbT = persist.tile([P, NK], DT, tag="k_sbT")
            q_sbT = persist.tile([P, NQ], DT, tag="q_sbT")
            v_sbT = persist.tile([P, NK], DT, tag="v_sbT")
            v_aug = persist.tile([P, NKT, DK + 1], DT, tag="v_aug")
            e_all = persist.tile([P, NKT, NQ], DT, tag="e_all")
            out_sbT = persist.tile([DK, NQ], F32, tag="out_sbT")
            onesr = persist.tile([1, DK], DT, tag="onesr")

            with (
                tc.tile_pool(name="xt", bufs=1) as xtp,
                tc.tile_pool(name="psout", bufs=1, space="PSUM") as pso,
            ):
                oacc = [
                    pso.tile([DK + 1, NQC], F32, tag=f"oacc{h}", name=f"oacc{h}")
                    for h in range(NCH_Q)
                ]
                # kq DMA stream: kt0, w3, qt0, kt1, qt1, kt2, kt3, b3
                kts, qts = [], []
                KT_GRAN = [(0, 3), (3, 3), (6, 1), (7, 1)]  # (dmt0, n_dmt)
                def dma_kt(i, eng):
                    d0, nd = KT_GRAN[i]
                    t_ = xtp.tile([P, nd, NK], DT, tag=f"kt{i}", name=f"kt{i}")
                    eng.dma_start(
                        t_[:], kT_d[d0 * P:(d0 + nd) * P, :].rearrange(
                            "(o p) n -> p o n", p=P))
                    kts.append(t_)
                def dma_qt(i, eng):
                    t_ = xtp.tile([P, 4, NQ], DT, tag=f"qt{i}", name=f"qt{i}")
                    eng.dma_start(
                        t_[:], qT_d[i * 4 * P:(i + 1) * 4 * P, :].rearrange(
                            "(o p) n -> p o n", p=P))
                    qts.append(t_)
                # alternate SP/ACT HWDGE queues so per-DMA setup overlaps the
                # serialized transfers (ACT is otherwise idle this early)
                dma_kt(0, nc.sync)
                nc.scalar.dma_start(
                    w3_sb[:], w3_d.rearrange("p (o k) -> p o k", o=NDM))
                nc.scalar.dma_start(b3_sb[:], b3_d[:])
                dma_qt(0, nc.scalar)
                dma_kt(1, nc.sync)
                dma_qt(1, nc.scalar)
                dma_kt(2, nc.sync)
                dma_kt(3, nc.scalar)

                make_identity(nc, ident[:])
                nc.vector.tensor_copy(identb[:], ident[:])
                # preload the exp table set while the DMA stream runs
                nc.scalar.activation(
                    e_all[0:1, 0, 0:1], ident[0:1, 0:1], EXP, scale=1.0)
                nc.gpsimd.memset(k_sbT[DK:P, :], 0.0)
                nc.gpsimd.memset(q_sbT[DK:P, :], 0.0)
                nc.gpsimd.memset(v_sbT[DK:P, :], 0.0)
                nc.gpsimd.memset(v_aug[:], 1.0)  # ones col at [:, :, 64]
                nc.gpsimd.memset(onesr[:], 1.0)

                # ---- k/q projections riding the DMA stream ----
                with tc.tile_pool(name="pskq", bufs=1, space="PSUM") as pskq:
                    psq = [pskq.tile([DK, NQC], F32, tag=f"psq{j}", name=f"psq{j}")
                           for j in range(NCH_Q)]
                    psk = [pskq.tile([DK, NQC], F32, tag=f"psk{j}", name=f"psk{j}")
                           for j in range(NCH_K)]

                    def kt_view(dmt):
                        for i, (d0, nd) in enumerate(KT_GRAN):
                            if d0 <= dmt < d0 + nd:
                                return kts[i][:, dmt - d0, :]
                        raise AssertionError(dmt)
                    def kp(dmts):
                        for dmt in dmts:
                            kv = kt_view(dmt)
                            for j in range(NCH_K):
                                nc.tensor.matmul(
                                    psk[j][:], w3_sb[:, dmt, DK:2 * DK],
                                    kv[:, j * NQC:(j + 1) * NQC],
                                    start=(dmt == 0), stop=(dmt == NDM - 1))
                    def qp(dmts):
                        for dmt in dmts:
                            for j in range(NCH_Q):
                                nc.tensor.matmul(
                                    psq[j][:], w3_sb[:, dmt, 0:DK],
                                    qts[dmt // 4][:, dmt % 4,
                                                  j * NQC:(j + 1) * NQC],
                                    start=(dmt == 0), stop=(dmt == NDM - 1))
                    kp([0])
                    kp([1, 2])
                    qp([0, 1, 2, 3])
                    kp([3, 4, 5])
                    # keep the PE warm while qt1/kt granules land; results
                    # are discarded (first real oacc matmul resets the bank)
                    for _w in range(14):
                        nc.tensor.matmul(
                            oacc[0][0:DK, 0:NQC], w3_sb[:, 0, 0:DK],
                            k_sbT[:, NK - NQC:NK], start=True, stop=True)
                    qp([4, 5, 6, 7])
                    kp([6])
                    kp([7])
                    # writebacks: k0,k1 on ACT; q0,q1,k2,k3 on DVE — so the
                    # psum banks reused by the scores pool free earliest
                    nc.scalar.activation(
                        k_sbT[0:DK, 0:NQC], psk[0][:], IDF, bias=b3_sb[:, 1:2])
                    nc.vector.tensor_scalar_add(
                        q_sbT[0:DK, 0:NQC], psq[0][:], b3_sb[:, 0:1])
                    nc.scalar.activation(
                        k_sbT[0:DK, NQC:2 * NQC], psk[1][:], IDF,
                        bias=b3_sb[:, 1:2])
                    nc.vector.tensor_scalar_add(
                        q_sbT[0:DK, NQC:2 * NQC], psq[1][:], b3_sb[:, 0:1])
                    nc.vector.tensor_scalar_add(
                        k_sbT[0:DK, 2 * NQC:3 * NQC], psk[2][:], b3_sb[:, 1:2])
                    nc.vector.tensor_scalar_add(
                        k_sbT[0:DK, 3 * NQC:4 * NQC], psk[3][:], b3_sb[:, 1:2])

                # ---- attention (t-major) with pipelined v chain ----
                # vT loaded chunk-major: granule j = all d_model for keys
                # [j*512, (j+1)*512); its projection, writeback, transposes
                # and the out-matmuls are interleaved into the scores/exp loop.
                vts = []
                for j in range(NCH_K):
                    vt = xtp.tile([P, NDM, NQC], DT, tag=f"vt{j}", name=f"vt{j}")
                    nc.sync.dma_start(
                        vt[:], vT_d[:, j * NQC:(j + 1) * NQC].rearrange(
                            "(o p) n -> p o n", p=P))
                    vts.append(vt)
                with (
                    tc.tile_pool(name="psscore", bufs=2, space="PSUM") as pss,
                    tc.tile_pool(name="psv", bufs=1, space="PSUM") as psvp,
                ):
                    psva = [None]

                    def v_mm(j, dmts):
                        if dmts[0] == 0:
                            psva[0] = psvp.tile(
                                [DK, NQC], F32, tag="psvacc", name=f"psva{j}")
                        for dmt in dmts:
                            nc.tensor.matmul(
                                psva[0][:], w3_sb[:, dmt, 2 * DK:3 * DK],
                                vts[j][:, dmt, :],
                                start=(dmt == 0), stop=(dmt == NDM - 1))
                    def v_wb(j):
                        nc.vector.tensor_scalar_add(
                            v_sbT[0:DK, j * NQC:(j + 1) * NQC], psva[0][:],
                            b3_sb[:, 2:3])
                    def v_tr(ts_):
                        for t_ in ts_:
                            pt = psvp.tile([P, P], DT, tag="psvb", name=f"pvb{t_}")
                            nc.tensor.transpose(
                                pt[:], v_sbT[:, t_ * P:(t_ + 1) * P], identb[:])
                            nc.vector.tensor_copy(v_aug[:, t_, 0:DK],
                                                  pt[:, 0:DK])
                    def o_mm(tp):
                        for h in range(NCH_Q):
                            nc.tensor.matmul(
                                oacc[h][:], v_aug[:, tp, :],
                                e_all[:, tp, h * NQC:(h + 1) * NQC],
                                start=(tp == 0), stop=(tp == NKT - 1))

                    # per-slot v-pipeline work: chunk j MMs at slots 4j+1/4j+2,
                    # writeback after, transposes at 4j+3/4j+4
                    vwork = {}
                    for j in range(NCH_K):
                        vwork.setdefault(2 * j + 1, []).append(
                            lambda j=j: v_mm(j, [0, 1, 2, 3]))
                        vwork.setdefault(2 * j + 2, []).append(
                            lambda j=j: (v_mm(j, [4, 5, 6, 7]), v_wb(j)))
                        vwork.setdefault(2 * j + 3, []).append(
                            lambda j=j: v_tr([4 * j, 4 * j + 1]))
                        vwork.setdefault(2 * j + 4, []).append(
                            lambda j=j: v_tr([4 * j + 2, 4 * j + 3]))

                    ODELAY = 5
                    def emit_scores(t):
                        sc = pss.tile([P, NQ], F32, tag="psscore",
                                      name=f"sc{t}")
                        for h in range(NCH_Q):
                            nc.tensor.matmul(
                                sc[:, h * NQC:(h + 1) * NQC],
                                k_sbT[:, t * P:(t + 1) * P],
                                q_sbT[:, h * NQC:(h + 1) * NQC],
                                start=True, stop=True)
                        return sc
                    # scores run one slot ahead of their exp so the per-slot
                    # v-chain/out-matmul work can never starve the ACT engine
                    sc_cur = emit_scores(0)
                    for t in range(NKT):
                        if t + 1 < NKT:
                            sc_next = emit_scores(t + 1)
                        nc.scalar.activation(
                            e_all[:, t, :], sc_cur[:], EXP, scale=SCALE)
                        if t + 1 < NKT:
                            sc_cur = sc_next
                        for fn in vwork.get(t, []):
                            fn()
                        if t >= ODELAY:
                            o_mm(t - ODELAY)
                    for fn in vwork.get(NKT, []):
                        fn()
                    for tp in range(NKT - ODELAY, NKT):
                        o_mm(tp)

                # ---- normalize in transposed layout + store ----
                # out^T[dk, nq] = oacc[0:64] * (1/oacc[64]) ; the reciprocal
                # row is broadcast across partitions with a K=1 matmul.
                with (
                    tc.tile_pool(name="fin", bufs=2) as fin,
                    tc.tile_pool(name="psfin", bufs=2, space="PSUM") as psf,
                ):
                    for h in range(NCH_Q):
                        rcr = fin.tile([1, NQC], F32, tag="rcr")
                        nc.vector.reciprocal(rcr[:], oacc[h][DK:DK + 1, :])
                        rcb = fin.tile([1, NQC], DT, tag="rcb")
                        nc.scalar.copy(rcb[:], rcr[:])
                        pb = psf.tile([DK, NQC], F32, tag="psfin")
                        nc.tensor.matmul(
                            pb[:], onesr[:], rcb[:], start=True, stop=True)
                        rcf = fin.tile([DK, NQC], F32, tag="rcf")
                        nc.scalar.copy(rcf[:], pb[:])
                        nc.vector.tensor_tensor(
                            out_sbT[:, h * NQC:(h + 1) * NQC],
                            oacc[h][0:DK, :], rcf[:], mybir.AluOpType.mult)
                        (nc.sync if h == 0 else nc.scalar).dma_start(
                            out_d[:, h * NQC:(h + 1) * NQC],
                            out_sbT[:, h * NQC:(h + 1) * NQC])
    _legalize_waits(nc)
    return nc


_nc_cache = None


def _get_nc():
    global _nc_cache
    if _nc_cache is None:
        _nc_cache = _build()
    return _nc_cache


def _marshal(q, k, v, Wq, bq, Wk, bk, Wv, bv):
    """Host-side layout prep: transpose to [B, d_model, N], cast to bf16,
    shard over (batch, query-half)."""
    qT = np.ascontiguousarray(np.transpose(np.asarray(q), (0, 2, 1))).astype(BF)
    kT = np.ascontiguousarray(np.transpose(np.asarray(k), (0, 2, 1))).astype(BF)
    vT = np.ascontiguousarray(np.transpose(np.asarray(v), (0, 2, 1))).astype(BF)
    w3 = np.concatenate(
        [np.asarray(Wq), np.asarray(Wk), np.asarray(Wv)], axis=1
    ).astype(BF)
    # [1024, 192] -> [128, 8*192] partition-major so the DMA is contiguous
    w3 = np.ascontiguousarray(
        w3.reshape(NDM, P, 3 * DK).transpose(1, 0, 2).reshape(P, NDM * 3 * DK)
    )
    b3 = np.stack(
        [np.asarray(bq), np.asarray(bk), np.asarray(bv)], axis=1
    ).astype(np.float32)
    in_maps = []
    for c in range(NCORES):
        bi, h = divmod(c, 2)
        in_maps.append({
            "qT": np.ascontiguousarray(qT[bi][:, h * NQ:(h + 1) * NQ]),
            "kT": kT[bi],
            "vT": vT[bi],
            "w3": w3, "b3": b3,
        })
    return in_maps


def _unmarshal(results):
    out = np.empty((B, N, DK), np.float32)
    for c in range(NCORES):
        bi, h = divmod(c, 2)
        out[bi, h * NQ:(h + 1) * NQ] = results[c]["out"].T
    return out


def kernel(q, k, v, Wq, bq, Wk, bk, Wv, bv):
    in_maps = _marshal(q, k, v, Wq, bq, Wk, bk, Wv, bv)
    res = run_bass_kernel_spmd(_get_nc(), in_maps, core_ids=list(range(NCORES)))
    return _unmarshal(res.results)



# revision 8
# speedup vs baseline: 1.4438x; 1.4438x over previous
"""Trainium2 Bass kernel for nn_AttentionHead (B=4, N=2048, d_model=1024, d_k=64).

Sharding: data-parallel over (batch, query-half) -> 8 cores. Each core gets
qT[b, :, h*1024:(h+1)*1024], full kT[b], vT[b] (host pre-transposes so d_model
lands on SBUF partitions), plus packed projection weights. Matmuls are bf16
with fp32 PSUM accumulation.

Per-core device program (ACT is the critical engine: 16 exp tiles):
  1. Chunk-major DMA granules of qT/kT stream on the SP/Pool/ACT queues in
     arrival-priority order (q first - scores need all of q_^T but only one
     key tile). ACT preloads the exp table right after its two granules so
     the exp stream starts the moment scores tile 0 lands.
  2. q_^T[64,1024] / k_^T[64,2048] projections accumulate per 512-chunk in
     two rotating PSUM banks (pj0/pj1); DVE writebacks fold the biases.
     Only q and k-chunk-0 gate the loop; k chunks 1-3 and the v chain are
     interleaved into the PE stream at slots where their granules and banks
     are ready.
  3. Main loop over 16 key tiles: scores^T tile [128,1024] in PSUM (2x2
     banks), one Exp ACTIVATE per tile (scale=1/sqrt(dk) folded) into a
     persistent e buffer. v chunks project non-transposed ([keys,64], no
     transposes) straight into v_aug[keys,16,65] whose column 64 is ones.
  4. Out-matmuls are query-major: oacc[qi][128q, 65] += e_t[:,qi]^T @ v_aug_t
     - only a 65-col moving dim on the PE; 8 accumulation groups packed into
     two PSUM banks with one start/stop per bank (zero-region semantics).
     Row 64 accumulates the softmax denominator via the ones column.
  5. No on-device normalize: oacc is written back and DMA'd out as [q,65];
     the host divides by column 64 and adds the v-bias (attn rows sum to 1).

A small legalization pass hoists excess per-instruction semaphore waits onto
same-engine NoOps (this container's walrus accepts at most one).
"""

import numpy as np
import ml_dtypes

import concourse.bass as bass
import concourse.tile as tile
from concourse import mybir
from concourse.bass_utils import run_bass_kernel_spmd

B, N, DM, DK = 4, 2048, 1024, 64
NCORES = 8
NQ = N // 2          # queries per core
NK = N               # keys per core
P = 128
NDM = DM // P        # 8 d_model tiles
NKT = NK // P        # 16 key tiles
NQT = NQ // P        # 8 query tiles
DT = mybir.dt.bfloat16
F32 = mybir.dt.float32
BF = ml_dtypes.bfloat16

ODELAY = 4           # out-matmul lag (tiles) behind the exp stream


# --- walrus wait legalization -------------------------------------------------
# The walrus build in this container accepts at most 1 sync wait + 1 sync
# update per instruction (2 for EventSemaphore). Excess WAITS are hoisted
# onto same-engine NoOps placed just before (queues issue in order, so the
# gating is preserved). Updates are completion-signals and stay put.

def _caps(inst):
    opcode = type(inst).__name__
    if opcode == "InstEventSemaphore":
        return 2, 2
    return 1, 1


def _legalize_waits(nc):
    for f in nc.m.functions:
        for bb in f.blocks:
            out = []
            changed = False
            for inst in bb.instructions:
                si = inst.sync_info
                waits = list(si.on_wait) if si is not None else []
                updates = list(si.on_update) if si is not None else []
                wcap, ucap = _caps(inst)
                if len(waits) <= wcap and len(updates) <= ucap:
                    out.append(inst)
                    continue
                changed = True
                keep_w = waits[len(waits) - wcap:] if wcap else []
                extra_w = waits[: len(waits) - wcap] if wcap else waits
                # Updates signal instruction COMPLETION (writes landed);
                # a following NoOp fires at issue time instead, which races
                # consumers against in-flight writes. Never hoist them.
                assert len(updates) <= ucap, (
                    f"{inst.name}: {len(updates)} sync updates exceed the "
                    f"per-instruction cap and cannot be hoisted safely"
                )
                for w in extra_w:
                    nop = mybir.InstNoOp(
                        name=nc.get_next_instruction_name(), ins=[], outs=[]
                    )
                    nop.engine = inst.engine
                    nop.sync_info = mybir.SyncInfo(on_wait=[w], on_update=[])
                    out.append(nop)
                inst.sync_info = mybir.SyncInfo(on_wait=keep_w, on_update=updates)
                out.append(inst)
            if changed:
                bb.instructions = out


# --- device program -----------------------------------------------------------

def _build(reps=1):
    nc = bass.Bass()
    qT_d = nc.dram_tensor("qT", [DM, NQ], DT, kind="ExternalInput")
    kT_d = nc.dram_tensor("kT", [DM, NK], DT, kind="ExternalInput")
    vT_d = nc.dram_tensor("vT", [DM, NK], DT, kind="ExternalInput")
    w3_d = nc.dram_tensor("w3", [P, NDM * 3 * DK], DT, kind="ExternalInput")
    b3_d = nc.dram_tensor("b3", [DK, 3], F32, kind="ExternalInput")
    out_d = nc.dram_tensor("out", [NQT, P, DK + 1], F32, kind="ExternalOutput")

    EXP = mybir.ActivationFunctionType.Exp
    SCALE = 1.0 / float(np.sqrt(np.float32(DK)))
    KGR = 256            # kT/qT granule width (keys / queries)
    VGR = 512            # vT granule width

    with tile.TileContext(nc) as tc:
      for _rep in range(reps):
        with tc.tile_pool(name="persist", bufs=1) as persist:
            w3_sb = persist.tile([P, NDM, 3 * DK], DT, tag="w3_sb")
            b3_sb = persist.tile([DK, 3], F32, tag="b3_sb")
            k_sbT = persist.tile([P, NK], DT, tag="k_sbT")
            q_sbT = persist.tile([P, NQ], DT, tag="q_sbT")
            v_aug = persist.tile([P, NKT, DK + 1], DT, tag="v_aug")
            e_all = persist.tile([P, NKT, NQ], DT, tag="e_all")
            out_sb = persist.tile([P, NQT, DK + 1], F32, tag="out_sb")

            with (
                tc.tile_pool(name="xt", bufs=1) as xtp,
                tc.tile_pool(name="pspj", bufs=1, space="PSUM") as pspj,
                tc.tile_pool(name="psscore", bufs=2, space="PSUM") as pss,
                tc.tile_pool(name="psout", bufs=1, space="PSUM") as pso,
            ):
                # ---- DMA streams -------------------------------------------
                # SP:   w3, q0, k0, k4, k5, v2, v3    (+ out half 0)
                # Pool: q1, q3, k2, v0, k3, v1, k6, k7 (+ out half 1)
                # ACT:  b3, q2, k1, exp table, then the exp stream.
                kgr, qgr, vgr = [None] * 8, [None] * 4, [None] * 4

                def dma_k(i, eng):
                    t_ = xtp.tile([P, NDM, KGR], DT, tag=f"kg{i}", name=f"kg{i}")
                    eng.dma_start(
                        t_[:], kT_d[:, i * KGR:(i + 1) * KGR].rearrange(
                            "(o p) n -> p o n", p=P))
                    kgr[i] = t_

                def dma_q(i, eng):
                    t_ = xtp.tile([P, NDM, KGR], DT, tag=f"qg{i}", name=f"qg{i}")
                    eng.dma_start(
                        t_[:], qT_d[:, i * KGR:(i + 1) * KGR].rearrange(
                            "(o p) n -> p o n", p=P))
                    qgr[i] = t_

                def dma_v(i, eng):
                    t_ = xtp.tile([P, NDM, VGR], DT, tag=f"vg{i}", name=f"vg{i}")
                    eng.dma_start(
                        t_[:], vT_d[:, i * VGR:(i + 1) * VGR].rearrange(
                            "(o p) n -> p o n", p=P))
                    vgr[i] = t_

                nc.sync.dma_start(
                    w3_sb[:], w3_d.rearrange("p (o k) -> p o k", o=NDM))
                nc.scalar.dma_start(b3_sb[:], b3_d[:])
                dma_q(1, nc.gpsimd)
                dma_q(2, nc.scalar)
                dma_q(0, nc.sync)
                dma_q(3, nc.gpsimd)
                dma_k(0, nc.sync)
                dma_k(1, nc.scalar)
                dma_k(2, nc.gpsimd)
                dma_k(4, nc.sync)
                dma_v(0, nc.gpsimd)
                dma_k(5, nc.sync)
                dma_k(3, nc.gpsimd)
                dma_v(1, nc.gpsimd)
                dma_v(2, nc.sync)
                dma_k(6, nc.gpsimd)
                dma_v(3, nc.sync)
                dma_k(7, nc.gpsimd)

                # preload the exp table while the DMA streams run
                nc.scalar.activation(
                    e_all[0:1, 0, 0:1], b3_sb[0:1, 0:1], EXP, scale=1.0)
                # zero-pad dead partitions (scores contract over 128; NaN
                # poison in uninitialized SBUF would survive 0*NaN)
                nc.vector.memset(q_sbT[DK:P, :], 0.0)
                nc.vector.memset(k_sbT[DK:P, :], 0.0)
                # ones column -> row 64 of out accumulates the denominator
                nc.vector.memset(v_aug[:, :, DK:DK + 1], 1.0)

                # ---- rotating projection / v-chain PSUM banks --------------
                def pjtile(which, name):
                    return pspj.tile([P, 512], F32, tag=f"pj{which}", name=name)

                oacc = [
                    pso.tile([P, 4, DK + 1], F32, tag=f"oacc{g}",
                             name=f"oacc{g}")
                    for g in range(2)
                ]

                def proj(ps, gr, half, wlo, start, stop):
                    for dmt in range(NDM):
                        nc.tensor.matmul(
                            ps[0:DK, half * KGR:(half + 1) * KGR],
                            w3_sb[:, dmt, wlo:wlo + DK],
                            gr[:, dmt, :],
                            start=(start and dmt == 0),
                            stop=(stop and dmt == NDM - 1))

                def wb_k(ps, c):
                    nc.vector.tensor_scalar_add(
                        k_sbT[0:DK, c * 512:(c + 1) * 512], ps[0:DK, :],
                        b3_sb[:, 1:2])

                def wb_q(ps, h):
                    nc.vector.tensor_scalar_add(
                        q_sbT[0:DK, h * 512:(h + 1) * 512], ps[0:DK, :],
                        b3_sb[:, 0:1])

                def v_chain(j):
                    # project v chunk j (4 key tiles, non-transposed) into
                    # v_aug; pv slots at 128-col strides, 64 cols used
                    pv = pjtile(0, f"psv{j}")
                    for dmt in range(NDM):
                        for kt in range(4):
                            nc.tensor.matmul(
                                pv[:, kt * P:kt * P + DK],
                                vgr[j][:, dmt, kt * P:(kt + 1) * P],
                                w3_sb[:, dmt, 2 * DK:3 * DK],
                                start=(dmt == 0 and kt == 0),
                                stop=(dmt == NDM - 1 and kt == 3))
                    nc.vector.tensor_copy(
                        v_aug[:, 4 * j:4 * j + 4, 0:DK],
                        pv[:].rearrange("p (k c) -> p k c", k=4)[:, :, 0:DK])

                def o_mm(t):
                    for qi in range(NQT):
                        nc.tensor.matmul(
                            oacc[qi // 4][:, qi % 4, :],
                            e_all[:, t, qi * P:(qi + 1) * P],
                            v_aug[:, t, :],
                            start=(t == 0 and qi % 4 == 0),
                            stop=(t == NKT - 1 and qi % 4 == 3))

                def emit_scores(t):
                    sc = pss.tile([P, NQ], F32, tag="psscore", name=f"sc{t}")
                    for h in range(2):
                        nc.tensor.matmul(
                            sc[:, h * 512:(h + 1) * 512],
                            k_sbT[:, t * P:(t + 1) * P],
                            q_sbT[:, h * 512:(h + 1) * 512],
                            start=True, stop=True)
                    return sc

                # ---- pre-loop: q projection + k chunk 0 --------------------
                # emission order tracks expected granule arrival
                psq0 = pjtile(0, "psq0")
                psq1 = pjtile(1, "psq1")
                proj(psq0, qgr[1], 1, 0, start=True, stop=False)
                proj(psq1, qgr[2], 0, 0, start=True, stop=False)
                proj(psq0, qgr[0], 0, 0, start=False, stop=True)
                wb_q(psq0, 0)
                proj(psq1, qgr[3], 1, 0, start=False, stop=True)
                wb_q(psq1, 1)
                psk0 = pjtile(0, "psk0")
                proj(psk0, kgr[1], 1, DK, start=True, stop=False)
                proj(psk0, kgr[0], 0, DK, start=False, stop=True)
                wb_k(psk0, 0)

                # ---- main loop: scores -> exp -> (k/v chains, out-mms) -----
                # per-slot extra PE work, keyed by loop slot index
                kwork = {}   # slot -> list of thunks

                def add(slot, fn):
                    kwork.setdefault(slot, []).append(fn)

                psk1 = [None]
                def mk_psk1_a():
                    psk1[0] = pjtile(1, "psk1")
                    proj(psk1[0], kgr[2], 0, DK, start=True, stop=False)
                def mk_psk1_b():
                    proj(psk1[0], kgr[3], 1, DK, start=False, stop=True)
                    wb_k(psk1[0], 1)
                psk2 = [None]
                def mk_psk2():
                    psk2[0] = pjtile(1, "psk2")
                    proj(psk2[0], kgr[4], 0, DK, start=True, stop=False)
                    proj(psk2[0], kgr[5], 1, DK, start=False, stop=True)
                    wb_k(psk2[0], 2)
                psk3 = [None]
                def mk_psk3_a():
                    psk3[0] = pjtile(1, "psk3")
                    proj(psk3[0], kgr[6], 0, DK, start=True, stop=False)
                def mk_psk3_b():
                    proj(psk3[0], kgr[7], 1, DK, start=False, stop=True)
                    wb_k(psk3[0], 3)

                add(0, mk_psk1_a)
                add(2, mk_psk1_b)
                add(2, lambda: v_chain(0))
                add(3, mk_psk2)
                add(5, lambda: v_chain(1))
                add(8, mk_psk3_a)
                add(9, mk_psk3_b)
                add(9, lambda: v_chain(2))
                add(12, lambda: v_chain(3))

                sc_cur = emit_scores(0)
                for t in range(NKT + ODELAY):
                    if t < NKT:
                        if t + 1 < NKT:
                            sc_next = emit_scores(t + 1)
                        nc.scalar.activation(
                            e_all[:, t, :], sc_cur[:], EXP, scale=SCALE)
                        if t + 1 < NKT:
                            sc_cur = sc_next
                    for fn in kwork.get(t, []):
                        fn()
                    if t >= ODELAY:
                        o_mm(t - ODELAY)

                # writeback + store (host does the softmax divide)
                for g in range(2):
                    nc.vector.tensor_copy(
                        out_sb[:, 4 * g:4 * g + 4, :], oacc[g][:])
                    (nc.sync if g == 0 else nc.gpsimd).dma_start(
                        out_d.rearrange("q p k -> p q k")[
                            :, 4 * g:4 * g + 4, :],
                        out_sb[:, 4 * g:4 * g + 4, :])
    _legalize_waits(nc)
    return nc


_nc_cache = None


def _get_nc():
    global _nc_cache
    if _nc_cache is None:
        _nc_cache = _build()
    return _nc_cache


def _marshal(q, k, v, Wq, bq, Wk, bk, Wv, bv):
    """Host-side layout prep: transpose to [B, d_model, N], cast to bf16,
    shard over (batch, query-half)."""
    qT = np.ascontiguousarray(np.transpose(np.asarray(q), (0, 2, 1))).astype(BF)
    kT = np.ascontiguousarray(np.transpose(np.asarray(k), (0, 2, 1))).astype(BF)
    vT = np.ascontiguousarray(np.transpose(np.asarray(v), (0, 2, 1))).astype(BF)
    w3 = np.concatenate(
        [np.asarray(Wq), np.asarray(Wk), np.asarray(Wv)], axis=1
    ).astype(BF)
    # [1024, 192] -> [128, 8*192] partition-major so the DMA is contiguous
    w3 = np.ascontiguousarray(
        w3.reshape(NDM, P, 3 * DK).transpose(1, 0, 2).reshape(P, NDM * 3 * DK)
    )
    b3 = np.stack(
        [np.asarray(bq), np.asarray(bk), np.asarray(bv)], axis=1
    ).astype(np.float32)
    in_maps = []
    for c in range(NCORES):
        bi, h = divmod(c, 2)
        in_maps.append({
            "qT": np.ascontiguousarray(qT[bi][:, h * NQ:(h + 1) * NQ]),
            "kT": kT[bi],
            "vT": vT[bi],
            "w3": w3, "b3": b3,
        })
    return in_maps


def _unmarshal(results, bv):
    out = np.empty((B, N, DK), np.float32)
    for c in range(NCORES):
        bi, h = divmod(c, 2)
        aug = results[c]["out"].reshape(NQ, DK + 1)   # [q, 65]
        out[bi, h * NQ:(h + 1) * NQ] = (
            aug[:, :DK] / aug[:, DK:DK + 1] + np.asarray(bv)[None, :]
        )
    return out


def kernel(q, k, v, Wq, bq, Wk, bk, Wv, bv):
    in_maps = _marshal(q, k, v, Wq, bq, Wk, bk, Wv, bv)
    res = run_bass_kernel_spmd(_get_nc(), in_maps, core_ids=list(range(NCORES)))
    return _unmarshal(res.results, bv)


# revision 24
# speedup vs baseline: 1.5049x; 1.0423x over previous
"""Trainium2 Bass kernel for nn_AttentionHead (B=4, N=2048, d_model=1024, d_k=64).

Sharding: data-parallel over (batch, query-half) -> 8 cores. Each core gets
qT[b, :, h*1024:(h+1)*1024], full kT[b], vT[b] (host pre-transposes so d_model
lands on SBUF partitions), plus packed projection weights. Matmuls are bf16
with fp32 PSUM accumulation.

Per-core device program (ACT is the critical engine: 16 exp tiles):
  1. Chunk-major DMA granules of qT/kT stream on the SP/Pool/ACT queues in
     arrival-priority order (q first - scores need all of q_^T but only one
     key tile). ACT preloads the exp table right after its two granules so
     the exp stream starts the moment scores tile 0 lands.
  2. q_^T[64,1024] / k_^T[64,2048] projections accumulate per 512-chunk in
     two rotating PSUM banks (pj0/pj1); DVE writebacks fold the biases.
     Only q and k-chunk-0 gate the loop; k chunks 1-3 and the v chain are
     interleaved into the PE stream at slots where their granules and banks
     are ready.
  3. Main loop over 16 key tiles: scores^T tile [128,1024] in PSUM (2x2
     banks), one Exp ACTIVATE per tile (scale=1/sqrt(dk) folded) into a
     persistent e buffer. v chunks project non-transposed ([keys,64], no
     transposes) straight into v_aug[keys,16,65] whose column 64 is ones.
  4. Out-matmuls are query-major: oacc[qi][128q, 65] += e_t[:,qi]^T @ v_aug_t
     - only a 65-col moving dim on the PE; 8 accumulation groups packed into
     two PSUM banks with one start/stop per bank (zero-region semantics).
     Row 64 accumulates the softmax denominator via the ones column.
  5. No on-device normalize: oacc is written back and DMA'd out as [q,65];
     the host divides by column 64 and adds the v-bias (attn rows sum to 1).

A small legalization pass hoists excess per-instruction semaphore waits onto
same-engine NoOps (this container's walrus accepts at most one).
"""

import numpy as np
import ml_dtypes

import concourse.bass as bass
import concourse.tile as tile
from concourse import mybir
from concourse.bass_utils import run_bass_kernel_spmd

B, N, DM, DK = 4, 2048, 1024, 64
NCORES = 8
NQ = N // 2          # queries per core
NK = N               # keys per core
P = 128
NDM = DM // P        # 8 d_model tiles
NKT = NK // P        # 16 key tiles
NQT = NQ // P        # 8 query tiles
DT = mybir.dt.bfloat16
F32 = mybir.dt.float32
BF = ml_dtypes.bfloat16

ODELAY = 4           # out-matmul lag (tiles) behind the exp stream


# --- walrus wait legalization -------------------------------------------------
# The walrus build in this container accepts at most 1 sync wait + 1 sync
# update per instruction (2 for EventSemaphore). Excess WAITS are hoisted
# onto same-engine NoOps placed just before (queues issue in order, so the
# gating is preserved). Updates are completion-signals and stay put.

def _caps(inst):
    opcode = type(inst).__name__
    if opcode == "InstEventSemaphore":
        return 2, 2
    return 1, 1


def _legalize_waits(nc):
    for f in nc.m.functions:
        for bb in f.blocks:
            out = []
            changed = False
            for inst in bb.instructions:
                si = inst.sync_info
                waits = list(si.on_wait) if si is not None else []
                updates = list(si.on_update) if si is not None else []
                wcap, ucap = _caps(inst)
                if len(waits) <= wcap and len(updates) <= ucap:
                    out.append(inst)
                    continue
                changed = True
                keep_w = waits[len(waits) - wcap:] if wcap else []
                extra_w = waits[: len(waits) - wcap] if wcap else waits
                # Updates signal instruction COMPLETION (writes landed);
                # a following NoOp fires at issue time instead, which races
                # consumers against in-flight writes. Never hoist them.
                assert len(updates) <= ucap, (
                    f"{inst.name}: {len(updates)} sync updates exceed the "
                    f"per-instruction cap and cannot be hoisted safely"
                )
                for w in extra_w:
                    nop = mybir.InstNoOp(
                        name=nc.get_next_instruction_name(), ins=[], outs=[]
                    )
                    nop.engine = inst.engine
                    nop.sync_info = mybir.SyncInfo(on_wait=[w], on_update=[])
                    out.append(nop)
                inst.sync_info = mybir.SyncInfo(on_wait=keep_w, on_update=updates)
                out.append(inst)
            if changed:
                bb.instructions = out


# --- device program -----------------------------------------------------------

def _build(reps=1):
    nc = bass.Bass()
    qT_d = nc.dram_tensor("qT", [DM, NQ], DT, kind="ExternalInput")
    kT_d = nc.dram_tensor("kT", [DM, NK], DT, kind="ExternalInput")
    vT_d = nc.dram_tensor("vT", [DM, NK], DT, kind="ExternalInput")
    w3_d = nc.dram_tensor("w3", [P, NDM * 3 * DK], DT, kind="ExternalInput")
    b3_d = nc.dram_tensor("b3", [DK, 3], F32, kind="ExternalInput")
    out_d = nc.dram_tensor("out", [P, NQT, DK + 1], F32, kind="ExternalOutput")

    EXP = mybir.ActivationFunctionType.Exp
    SCALE = 1.0 / float(np.sqrt(np.float32(DK)))
    QGR = 256            # qT granule width
    VGR = 512            # vT granule width

    with tile.TileContext(nc) as tc:
      for _rep in range(reps):
        with tc.tile_pool(name="persist", bufs=1) as persist:
            w3_sb = persist.tile([P, NDM, 3 * DK], DT, tag="w3_sb")
            b3_sb = persist.tile([DK, 3], F32, tag="b3_sb")
            k_sbT = persist.tile([P, NK], DT, tag="k_sbT")
            q_sbT = persist.tile([P, NQ], DT, tag="q_sbT")
            v_aug = persist.tile([P, NKT, DK + 1], DT, tag="v_aug")
            e_all = persist.tile([P, NKT, NQ], DT, tag="e_all")
            out_sb = persist.tile([P, NQT, DK + 1], F32, tag="out_sb")

            with (
                tc.tile_pool(name="xt", bufs=1) as xtp,
                tc.tile_pool(name="pspj", bufs=1, space="PSUM") as pspj,
                tc.tile_pool(name="psscore", bufs=2, space="PSUM") as pss,
                tc.tile_pool(name="psout", bufs=1, space="PSUM") as pso,
            ):
                # ---- DMA streams -------------------------------------------
                # k granule 0a = key tile 0 alone (2KB, lands ~2.6us on ACT)
                # so scores tile 0 unblocks as soon as q_^T completes.
                # SP:   w3, q0, k0b, k2, v0, v2      (+ out half 0)
                # Pool: q1, q3, k1, k3, v1, v3      (+ out half 1)
                # ACT:  b3, k0a, q2, exp table, then the exp stream.
                kgr = {}
                qgr, vgr = [None] * 4, [None] * 4

                def dma_k(nm, lo, hi, eng):
                    t_ = xtp.tile([P, NDM, hi - lo], DT, tag=f"kg{nm}",
                                  name=f"kg{nm}")
                    eng.dma_start(
                        t_[:], kT_d[:, lo:hi].rearrange(
                            "(o p) n -> p o n", p=P))
                    kgr[nm] = t_

                def dma_q(i, eng):
                    t_ = xtp.tile([P, NDM, QGR], DT, tag=f"qg{i}", name=f"qg{i}")
                    eng.dma_start(
                        t_[:], qT_d[:, i * QGR:(i + 1) * QGR].rearrange(
                            "(o p) n -> p o n", p=P))
                    qgr[i] = t_

                def dma_v(i, eng):
                    t_ = xtp.tile([P, NDM, VGR], DT, tag=f"vg{i}", name=f"vg{i}")
                    eng.dma_start(
                        t_[:], vT_d[:, i * VGR:(i + 1) * VGR].rearrange(
                            "(o p) n -> p o n", p=P))
                    vgr[i] = t_

                nc.sync.dma_start(
                    w3_sb[:], w3_d.rearrange("p (o k) -> p o k", o=NDM))
                nc.scalar.dma_start(b3_sb[:], b3_d[:])
                dma_q(1, nc.gpsimd)
                dma_k("0a", 0, 128, nc.scalar)
                dma_q(0, nc.sync)
                dma_q(3, nc.gpsimd)
                dma_q(2, nc.scalar)
                dma_k("0b", 128, 512, nc.sync)
                dma_k("1", 512, 1024, nc.gpsimd)
                dma_v(0, nc.sync)
                dma_k("3", 1536, 2048, nc.gpsimd)
                dma_k("2", 1024, 1536, nc.sync)
                dma_v(1, nc.gpsimd)
                dma_v(2, nc.sync)
                dma_v(3, nc.gpsimd)

                # preload the exp table while the DMA streams run
                nc.scalar.activation(
                    e_all[0:1, 0, 0:1], b3_sb[0:1, 0:1], EXP, scale=1.0)
                # zero-pad dead partitions (scores contract over 128; NaN
                # poison in uninitialized SBUF would survive 0*NaN)
                nc.vector.memset(q_sbT[DK:P, :], 0.0)
                nc.vector.memset(k_sbT[DK:P, :], 0.0)
                # ones column -> row 64 of out accumulates the denominator
                nc.vector.memset(v_aug[:, :, DK:DK + 1], 1.0)

                # ---- rotating projection / v-chain PSUM banks --------------
                def pjtile(which, name):
                    return pspj.tile([P, 512], F32, tag=f"pj{which}", name=name)

                oacc = [
                    pso.tile([P, 4, DK + 1], F32, tag=f"oacc{g}",
                             name=f"oacc{g}")
                    for g in range(2)
                ]

                def proj(ps, gr, off, wid, wlo, start, stop, glo=0):
                    # accumulate granule cols [glo, glo+wid) into psum bank
                    # cols [off, off+wid)
                    for dmt in range(NDM):
                        nc.tensor.matmul(
                            ps[0:DK, off:off + wid],
                            w3_sb[:, dmt, wlo:wlo + DK],
                            gr[:, dmt, glo:glo + wid],
                            start=(start and dmt == 0),
                            stop=(stop and dmt == NDM - 1))

                def wb_k(ps, off, wid, dst):
                    nc.vector.tensor_scalar_add(
                        k_sbT[0:DK, dst:dst + wid], ps[0:DK, off:off + wid],
                        b3_sb[:, 1:2])

                def wb_q(ps, off, wid, dst):
                    nc.vector.tensor_scalar_add(
                        q_sbT[0:DK, dst:dst + wid], ps[0:DK, off:off + wid],
                        b3_sb[:, 0:1])

                def v_mm(pv, j, klo, khi, start, stop):
                    # project v chunk-j key tiles [klo,khi) (global indices,
                    # non-transposed); tile kt sits at pv cols (kt%4)*128
                    for dmt in range(NDM):
                        for kt in range(klo, khi):
                            lo = (kt - 4 * j) * P
                            nc.tensor.matmul(
                                pv[:, lo:lo + DK],
                                vgr[j][:, dmt, lo:lo + P],
                                w3_sb[:, dmt, 2 * DK:3 * DK],
                                start=(start and dmt == 0 and kt == klo),
                                stop=(stop and dmt == NDM - 1
                                      and kt == khi - 1))

                def v_wb(pv, j, klo, khi):
                    src = pv[:].rearrange("p (k c) -> p k c", c=P)
                    nc.vector.tensor_copy(
                        v_aug[:, klo:khi, 0:DK],
                        src[:, klo - 4 * j:khi - 4 * j, 0:DK])

                def o_mm(t):
                    for qi in range(NQT):
                        nc.tensor.matmul(
                            oacc[qi // 4][:, qi % 4, :],
                            e_all[:, t, qi * P:(qi + 1) * P],
                            v_aug[:, t, :],
                            start=(t == 0 and qi % 4 == 0),
                            stop=(t == NKT - 1 and qi % 4 == 3))

                def emit_scores(t):
                    sc = pss.tile([P, NQ], F32, tag="psscore", name=f"sc{t}")
                    for h in range(2):
                        nc.tensor.matmul(
                            sc[:, h * 512:(h + 1) * 512],
                            k_sbT[:, t * P:(t + 1) * P],
                            q_sbT[:, h * 512:(h + 1) * 512],
                            start=True, stop=True)
                    return sc

                # ---- pre-loop: q projection + k tile 0 ---------------------
                # emission order tracks expected granule arrival; per-granule
                # writebacks keep the critical chains short. k tiles 0-3
                # accumulate in the first scores buffer (idle until sc1).
                psk0 = pss.tile([P, NQ], F32, tag="psscore", name="psk0")
                proj(psk0, kgr["0a"], 0, 128, DK, start=True, stop=False)
                psq0 = pjtile(0, "psq0")
                psq1 = pjtile(1, "psq1")
                proj(psq0, qgr[1], 256, 256, 0, start=True, stop=False)
                wb_q(psq0, 256, 256, 256)
                proj(psq0, qgr[0], 0, 256, 0, start=False, stop=True)
                wb_q(psq0, 0, 256, 0)
                proj(psq1, qgr[2], 0, 256, 0, start=True, stop=False)
                wb_q(psq1, 0, 256, 512)
                proj(psq1, qgr[3], 256, 256, 0, start=False, stop=True)
                wb_q(psq1, 256, 256, 768)
                wb_k(psk0, 0, 128, 0)

                # ---- main loop: scores -> exp -> (k/v chains, out-mms) -----
                # per-slot extra PE work, keyed by loop slot index
                kwork = {}   # slot -> list of thunks

                def add(slot, fn):
                    kwork.setdefault(slot, []).append(fn)

                def mk(fn):          # bind loop vars eagerly
                    return fn

                psk_t = [None]
                def psk_mm(nm, c, half, start, stop, wb):
                    if psk_t[0] is None or start:
                        psk_t[0] = pjtile(1, f"psk{c}")
                    ps = psk_t[0]
                    proj(ps, kgr[nm], half * 256, 256, DK,
                         start=start, stop=stop, glo=half * 256)
                    if wb:
                        wb_k(ps, 0, 512, c * 512)

                psv_t = [None]
                def psv_mm(j, klo, khi, start, stop):
                    if start:
                        psv_t[0] = pjtile(0, f"psv{j}_{klo}")
                    pv = psv_t[0]
                    v_mm(pv, j, klo, khi, start, stop)
                    v_wb(pv, j, klo, khi)

                # kg1/kg2/kg3 are full 512-wide granules: 16 mm each, split
                # across two slots (8 mm per slot) to avoid starving scores
                for c, nm, s0 in ((1, "1", 0), (2, "2", 4), (3, "3", 7)):
                    add(s0, mk(lambda nm=nm, c=c:
                               psk_mm(nm, c, 0, True, False, False)))
                    add(s0 + 1, mk(lambda nm=nm, c=c:
                                   psk_mm(nm, c, 1, False, True, True)))
                # v sub-chains, two key tiles at a time
                add(2, mk(lambda: psv_mm(0, 0, 2, True, False)))
                add(5, mk(lambda: psv_mm(0, 2, 4, False, True)))
                add(8, mk(lambda: psv_mm(1, 4, 6, True, False)))
                add(9, mk(lambda: psv_mm(1, 6, 8, False, True)))
                add(10, mk(lambda: psv_mm(2, 8, 10, True, False)))
                add(11, mk(lambda: psv_mm(2, 10, 12, False, True)))
                add(12, mk(lambda: psv_mm(3, 12, 14, True, False)))
                add(13, mk(lambda: psv_mm(3, 14, 16, False, True)))

                sc_cur = emit_scores(0)
                # k chunk 0 tiles 1-3 (granule k0b) finish in psk0; this must
                # precede sc1, which rotates back onto psk0's buffer
                proj(psk0, kgr["0b"], 128, 384, DK, start=False, stop=True)
                wb_k(psk0, 128, 384, 128)
                for t in range(NKT + ODELAY):
                    if t < NKT:
                        if t + 1 < NKT:
                            sc_next = emit_scores(t + 1)
                        nc.scalar.activation(
                            e_all[:, t, :], sc_cur[:], EXP, scale=SCALE)
                        if t + 1 < NKT:
                            sc_cur = sc_next
                    for fn in kwork.get(t, []):
                        fn()
                    if t >= ODELAY:
                        o_mm(t - ODELAY)

                # writeback + store (host does the softmax divide); the two
                # halves write back on DVE and ACT in parallel, and the
                # partition-major out layout gives one 1KB descriptor per
                # partition per DMA
                nc.vector.tensor_copy(out_sb[:, 0:4, :], oacc[0][:])
                nc.scalar.copy(out_sb[:, 4:8, :], oacc[1][:])
                nc.sync.dma_start(out_d[:, 0:4, :], out_sb[:, 0:4, :])
                nc.gpsimd.dma_start(out_d[:, 4:8, :], out_sb[:, 4:8, :])
    _legalize_waits(nc)
    return nc


_nc_cache = None


def _get_nc():
    global _nc_cache
    if _nc_cache is None:
        _nc_cache = _build()
    return _nc_cache


def _marshal(q, k, v, Wq, bq, Wk, bk, Wv, bv):
    """Host-side layout prep: transpose to [B, d_model, N], cast to bf16,
    shard over (batch, query-half)."""
    qT = np.ascontiguousarray(np.transpose(np.asarray(q), (0, 2, 1))).astype(BF)
    kT = np.ascontiguousarray(np.transpose(np.asarray(k), (0, 2, 1))).astype(BF)
    vT = np.ascontiguousarray(np.transpose(np.asarray(v), (0, 2, 1))).astype(BF)
    w3 = np.concatenate(
        [np.asarray(Wq), np.asarray(Wk), np.asarray(Wv)], axis=1
    ).astype(BF)
    # [1024, 192] -> [128, 8*192] partition-major so the DMA is contiguous
    w3 = np.ascontiguousarray(
        w3.reshape(NDM, P, 3 * DK).transpose(1, 0, 2).reshape(P, NDM * 3 * DK)
    )
    b3 = np.stack(
        [np.asarray(bq), np.asarray(bk), np.asarray(bv)], axis=1
    ).astype(np.float32)
    in_maps = []
    for c in range(NCORES):
        bi, h = divmod(c, 2)
        in_maps.append({
            "qT": np.ascontiguousarray(qT[bi][:, h * NQ:(h + 1) * NQ]),
            "kT": kT[bi],
            "vT": vT[bi],
            "w3": w3, "b3": b3,
        })
    return in_maps


def _unmarshal(results, bv):
    out = np.empty((B, N, DK), np.float32)
    for c in range(NCORES):
        bi, h = divmod(c, 2)
        aug = np.transpose(results[c]["out"], (1, 0, 2)).reshape(NQ, DK + 1)
        out[bi, h * NQ:(h + 1) * NQ] = (
            aug[:, :DK] / aug[:, DK:DK + 1] + np.asarray(bv)[None, :]
        )
    return out


def kernel(q, k, v, Wq, bq, Wk, bk, Wv, bv):
    in_maps = _marshal(q, k, v, Wq, bq, Wk, bk, Wv, bv)
    res = run_bass_kernel_spmd(_get_nc(), in_maps, core_ids=list(range(NCORES)))
    return _unmarshal(res.results, bv)


# revision 27
# speedup vs baseline: 1.5125x; 1.0051x over previous
"""Trainium2 Bass kernel for nn_AttentionHead (B=4, N=2048, d_model=1024, d_k=64).

Sharding: data-parallel over (batch, query-half) -> 8 cores. Each core gets
qT[b, :, h*1024:(h+1)*1024], full kT[b], vT[b] (host pre-transposes so d_model
lands on SBUF partitions), plus packed projection weights. Matmuls are bf16
with fp32 PSUM accumulation.

Per-core device program (ACT is the critical engine: 16 exp tiles):
  1. Chunk-major DMA granules of qT/kT stream on the SP/Pool/ACT queues in
     arrival-priority order (q first - scores need all of q_^T but only one
     key tile). ACT preloads the exp table right after its two granules so
     the exp stream starts the moment scores tile 0 lands.
  2. q_^T[64,1024] / k_^T[64,2048] projections accumulate per 512-chunk in
     two rotating PSUM banks (pj0/pj1); DVE writebacks fold the biases.
     Only q and k-chunk-0 gate the loop; k chunks 1-3 and the v chain are
     interleaved into the PE stream at slots where their granules and banks
     are ready.
  3. Main loop over 16 key tiles: scores^T tile [128,1024] in PSUM (2x2
     banks), one Exp ACTIVATE per tile (scale=1/sqrt(dk) folded) into a
     persistent e buffer. v chunks project non-transposed ([keys,64], no
     transposes) straight into v_aug[keys,16,65] whose column 64 is ones.
  4. Out-matmuls are query-major: oacc[qi][128q, 65] += e_t[:,qi]^T @ v_aug_t
     - only a 65-col moving dim on the PE; 8 accumulation groups packed into
     two PSUM banks with one start/stop per bank (zero-region semantics).
     Row 64 accumulates the softmax denominator via the ones column.
  5. No on-device normalize: oacc is written back and DMA'd out as [q,65];
     the host divides by column 64 and adds the v-bias (attn rows sum to 1).

A small legalization pass hoists excess per-instruction semaphore waits onto
same-engine NoOps (this container's walrus accepts at most one).
"""

import numpy as np
import ml_dtypes

import concourse.bass as bass
import concourse.tile as tile
from concourse import mybir
from concourse.bass_utils import run_bass_kernel_spmd

B, N, DM, DK = 4, 2048, 1024, 64
NCORES = 8
NQ = N // 2          # queries per core
NK = N               # keys per core
P = 128
NDM = DM // P        # 8 d_model tiles
NKT = NK // P        # 16 key tiles
NQT = NQ // P        # 8 query tiles
DT = mybir.dt.bfloat16
F32 = mybir.dt.float32
BF = ml_dtypes.bfloat16

ODELAY = 4           # out-matmul lag (tiles) behind the exp stream


# --- walrus wait legalization -------------------------------------------------
# The walrus build in this container accepts at most 1 sync wait + 1 sync
# update per instruction (2 for EventSemaphore). Excess WAITS are hoisted
# onto same-engine NoOps placed just before (queues issue in order, so the
# gating is preserved). Updates are completion-signals and stay put.

def _caps(inst):
    opcode = type(inst).__name__
    if opcode == "InstEventSemaphore":
        return 2, 2
    return 1, 1


def _legalize_waits(nc):
    for f in nc.m.functions:
        for bb in f.blocks:
            out = []
            changed = False
            for inst in bb.instructions:
                si = inst.sync_info
                waits = list(si.on_wait) if si is not None else []
                updates = list(si.on_update) if si is not None else []
                wcap, ucap = _caps(inst)
                if len(waits) <= wcap and len(updates) <= ucap:
                    out.append(inst)
                    continue
                changed = True
                keep_w = waits[len(waits) - wcap:] if wcap else []
                extra_w = waits[: len(waits) - wcap] if wcap else waits
                # Updates signal instruction COMPLETION (writes landed);
                # a following NoOp fires at issue time instead, which races
                # consumers against in-flight writes. Never hoist them.
                assert len(updates) <= ucap, (
                    f"{inst.name}: {len(updates)} sync updates exceed the "
                    f"per-instruction cap and cannot be hoisted safely"
                )
                for w in extra_w:
                    nop = mybir.InstNoOp(
                        name=nc.get_next_instruction_name(), ins=[], outs=[]
                    )
                    nop.engine = inst.engine
                    nop.sync_info = mybir.SyncInfo(on_wait=[w], on_update=[])
                    out.append(nop)
                inst.sync_info = mybir.SyncInfo(on_wait=keep_w, on_update=updates)
                out.append(inst)
            if changed:
                bb.instructions = out


# --- device program -----------------------------------------------------------

def _build(reps=1):
    nc = bass.Bass()
    qT_d = nc.dram_tensor("qT", [DM, NQ], DT, kind="ExternalInput")
    kT_d = nc.dram_tensor("kT", [DM, NK], DT, kind="ExternalInput")
    vT_d = nc.dram_tensor("vT", [DM, NK], DT, kind="ExternalInput")
    w3_d = nc.dram_tensor("w3", [P, NDM * 3 * DK], DT, kind="ExternalInput")
    b3_d = nc.dram_tensor("b3", [DK, 3], F32, kind="ExternalInput")
    out_d = nc.dram_tensor("out", [P, NQT, DK + 1], F32, kind="ExternalOutput")

    EXP = mybir.ActivationFunctionType.Exp
    SCALE = 1.0 / float(np.sqrt(np.float32(DK)))
    QGR = 256            # qT granule width
    VGR = 512            # vT granule width

    with tile.TileContext(nc) as tc:
      for _rep in range(reps):
        with tc.tile_pool(name="persist", bufs=1) as persist:
            w3_sb = persist.tile([P, NDM, 3 * DK], DT, tag="w3_sb")
            b3_sb = persist.tile([DK, 3], F32, tag="b3_sb")
            k_sbT = persist.tile([P, NK], DT, tag="k_sbT")
            q_sbT = persist.tile([P, NQ], DT, tag="q_sbT")
            v_aug = persist.tile([P, NKT, DK + 1], DT, tag="v_aug")
            e_all = persist.tile([P, NKT, NQ], DT, tag="e_all")
            out_sb = persist.tile([P, NQT, DK + 1], F32, tag="out_sb")

            with (
                tc.tile_pool(name="xt", bufs=1) as xtp,
                tc.tile_pool(name="pspj", bufs=1, space="PSUM") as pspj,
                tc.tile_pool(name="psscore", bufs=2, space="PSUM") as pss,
                tc.tile_pool(name="psout", bufs=1, space="PSUM") as pso,
            ):
                # ---- DMA streams -------------------------------------------
                # k granule 0a = key tile 0 alone (2KB, lands ~2.6us on ACT)
                # so scores tile 0 unblocks as soon as q_^T completes.
                # SP:   w3, q0, k0b, k2, v0, v2      (+ out half 0)
                # Pool: q1, q3, k1, k3, v1, v3      (+ out half 1)
                # ACT:  b3, k0a, q2, exp table, then the exp stream.
                kgr = {}
                qgr, vgr = [None] * 4, [None] * 4

                def dma_k(nm, lo, hi, eng):
                    t_ = xtp.tile([P, NDM, hi - lo], DT, tag=f"kg{nm}",
                                  name=f"kg{nm}")
                    eng.dma_start(
                        t_[:], kT_d[:, lo:hi].rearrange(
                            "(o p) n -> p o n", p=P))
                    kgr[nm] = t_

                def dma_q(i, eng):
                    t_ = xtp.tile([P, NDM, QGR], DT, tag=f"qg{i}", name=f"qg{i}")
                    eng.dma_start(
                        t_[:], qT_d[:, i * QGR:(i + 1) * QGR].rearrange(
                            "(o p) n -> p o n", p=P))
                    qgr[i] = t_

                def dma_v(i, eng):
                    t_ = xtp.tile([P, NDM, VGR], DT, tag=f"vg{i}", name=f"vg{i}")
                    eng.dma_start(
                        t_[:], vT_d[:, i * VGR:(i + 1) * VGR].rearrange(
                            "(o p) n -> p o n", p=P))
                    vgr[i] = t_

                nc.sync.dma_start(
                    w3_sb[:], w3_d.rearrange("p (o k) -> p o k", o=NDM))
                nc.scalar.dma_start(b3_sb[:], b3_d[:])
                dma_q(1, nc.gpsimd)
                dma_k("0a", 0, 256, nc.scalar)
                dma_q(0, nc.sync)
                dma_q(3, nc.gpsimd)
                dma_q(2, nc.scalar)
                dma_k("0b", 256, 512, nc.sync)
                dma_k("1", 512, 1024, nc.gpsimd)
                dma_v(0, nc.sync)
                dma_k("3", 1536, 2048, nc.gpsimd)
                dma_k("2", 1024, 1536, nc.sync)
                dma_v(1, nc.gpsimd)
                dma_v(2, nc.sync)
                dma_v(3, nc.gpsimd)

                # preload the exp table while the DMA streams run
                nc.scalar.activation(
                    e_all[0:1, 0, 0:1], b3_sb[0:1, 0:1], EXP, scale=1.0)
                # zero-pad dead partitions (scores contract over 128; NaN
                # poison in uninitialized SBUF would survive 0*NaN)
                nc.vector.memset(q_sbT[DK:P, :], 0.0)
                nc.vector.memset(k_sbT[DK:P, :], 0.0)
                # ones column -> row 64 of out accumulates the denominator
                nc.vector.memset(v_aug[:, :, DK:DK + 1], 1.0)

                # ---- rotating projection / v-chain PSUM banks --------------
                def pjtile(which, name):
                    return pspj.tile([P, 512], F32, tag=f"pj{which}", name=name)

                oacc = [
                    pso.tile([P, 4, DK + 1], F32, tag=f"oacc{g}",
                             name=f"oacc{g}")
                    for g in range(2)
                ]

                def proj(ps, gr, off, wid, wlo, start, stop, glo=0):
                    # accumulate granule cols [glo, glo+wid) into psum bank
                    # cols [off, off+wid)
                    for dmt in range(NDM):
                        nc.tensor.matmul(
                            ps[0:DK, off:off + wid],
                            w3_sb[:, dmt, wlo:wlo + DK],
                            gr[:, dmt, glo:glo + wid],
                            start=(start and dmt == 0),
                            stop=(stop and dmt == NDM - 1))

                def wb_k(ps, off, wid, dst):
                    nc.vector.tensor_scalar_add(
                        k_sbT[0:DK, dst:dst + wid], ps[0:DK, off:off + wid],
                        b3_sb[:, 1:2])

                def wb_q(ps, off, wid, dst):
                    nc.vector.tensor_scalar_add(
                        q_sbT[0:DK, dst:dst + wid], ps[0:DK, off:off + wid],
                        b3_sb[:, 0:1])

                def v_mm(pv, j, klo, khi, start, stop):
                    # project v chunk-j key tiles [klo,khi) (global indices,
                    # non-transposed); tile kt sits at pv cols (kt%4)*128
                    for dmt in range(NDM):
                        for kt in range(klo, khi):
                            lo = (kt - 4 * j) * P
                            nc.tensor.matmul(
                                pv[:, lo:lo + DK],
                                vgr[j][:, dmt, lo:lo + P],
                                w3_sb[:, dmt, 2 * DK:3 * DK],
                                start=(start and dmt == 0 and kt == klo),
                                stop=(stop and dmt == NDM - 1
                                      and kt == khi - 1))

                def v_wb(pv, j, klo, khi):
                    src = pv[:].rearrange("p (k c) -> p k c", c=P)
                    nc.vector.tensor_copy(
                        v_aug[:, klo:khi, 0:DK],
                        src[:, klo - 4 * j:khi - 4 * j, 0:DK])

                def o_mm(t):
                    for qi in range(NQT):
                        nc.tensor.matmul(
                            oacc[qi // 4][:, qi % 4, :],
                            e_all[:, t, qi * P:(qi + 1) * P],
                            v_aug[:, t, :],
                            start=(t == 0 and qi % 4 == 0),
                            stop=(t == NKT - 1 and qi % 4 == 3))

                def emit_scores(t):
                    sc = pss.tile([P, NQ], F32, tag="psscore", name=f"sc{t}")
                    for h in range(2):
                        nc.tensor.matmul(
                            sc[:, h * 512:(h + 1) * 512],
                            k_sbT[:, t * P:(t + 1) * P],
                            q_sbT[:, h * 512:(h + 1) * 512],
                            start=True, stop=True)
                    return sc

                # ---- pre-loop: q projection + k tile 0 ---------------------
                # emission order tracks expected granule arrival; per-granule
                # writebacks keep the critical chains short. k tiles 0-3
                # accumulate in the first scores buffer (idle until sc1).
                psk0 = pss.tile([P, NQ], F32, tag="psscore", name="psk0")
                psq0 = pjtile(0, "psq0")
                psq1 = pjtile(1, "psq1")
                proj(psq0, qgr[1], 256, 256, 0, start=True, stop=False)
                proj(psq0, qgr[0], 0, 256, 0, start=False, stop=True)
                wb_q(psq0, 0, 512, 0)
                proj(psk0, kgr["0a"], 0, 256, DK, start=True, stop=False)
                wb_k(psk0, 0, 256, 0)
                proj(psq1, qgr[2], 0, 256, 0, start=True, stop=False)
                proj(psq1, qgr[3], 256, 256, 0, start=False, stop=True)
                wb_q(psq1, 0, 512, 512)

                # ---- main loop: scores -> exp -> (k/v chains, out-mms) -----
                # per-slot extra PE work, keyed by loop slot index
                kwork = {}   # slot -> list of thunks

                def add(slot, fn):
                    kwork.setdefault(slot, []).append(fn)

                def mk(fn):          # bind loop vars eagerly
                    return fn

                psk_t = [None]
                def psk_mm(nm, c, half, start, stop, wb):
                    if psk_t[0] is None or start:
                        psk_t[0] = pjtile(1, f"psk{c}")
                    ps = psk_t[0]
                    proj(ps, kgr[nm], half * 256, 256, DK,
                         start=start, stop=stop, glo=half * 256)
                    if wb:
                        wb_k(ps, 0, 512, c * 512)

                psv_t = [None]
                def psv_mm(j, klo, khi, start, stop):
                    if start:
                        psv_t[0] = pjtile(0, f"psv{j}_{klo}")
                    pv = psv_t[0]
                    v_mm(pv, j, klo, khi, start, stop)
                    v_wb(pv, j, klo, khi)

                # kg1/kg2/kg3 are full 512-wide granules: 16 mm each, split
                # across two slots (8 mm per slot) to avoid starving scores
                for c, nm, s0 in ((1, "1", 0), (2, "2", 4), (3, "3", 7)):
                    add(s0, mk(lambda nm=nm, c=c:
                               psk_mm(nm, c, 0, True, False, False)))
                    add(s0 + 1, mk(lambda nm=nm, c=c:
                                   psk_mm(nm, c, 1, False, True, True)))
                # v sub-chains, two key tiles at a time
                add(2, mk(lambda: psv_mm(0, 0, 2, True, False)))
                add(5, mk(lambda: psv_mm(0, 2, 4, False, True)))
                add(8, mk(lambda: psv_mm(1, 4, 6, True, False)))
                add(9, mk(lambda: psv_mm(1, 6, 8, False, True)))
                add(10, mk(lambda: psv_mm(2, 8, 10, True, False)))
                add(11, mk(lambda: psv_mm(2, 10, 12, False, True)))
                add(12, mk(lambda: psv_mm(3, 12, 14, True, False)))
                add(13, mk(lambda: psv_mm(3, 14, 16, False, True)))

                sc_cur = emit_scores(0)
                # k chunk 0 tiles 2-3 (granule k0b) finish in psk0; this must
                # precede sc1, which rotates back onto psk0's buffer
                proj(psk0, kgr["0b"], 256, 256, DK, start=False, stop=True)
                wb_k(psk0, 256, 256, 256)
                for t in range(NKT + ODELAY):
                    if t < NKT:
                        if t + 1 < NKT:
                            sc_next = emit_scores(t + 1)
                        nc.scalar.activation(
                            e_all[:, t, :], sc_cur[:], EXP, scale=SCALE)
                        if t + 1 < NKT:
                            sc_cur = sc_next
                    for fn in kwork.get(t, []):
                        fn()
                    if t >= ODELAY:
                        o_mm(t - ODELAY)

                # writeback + store (host does the softmax divide); the two
                # halves write back on DVE and ACT in parallel, and the
                # partition-major out layout gives one 1KB descriptor per
                # partition per DMA
                nc.vector.tensor_copy(out_sb[:, 0:4, :], oacc[0][:])
                nc.scalar.copy(out_sb[:, 4:8, :], oacc[1][:])
                nc.sync.dma_start(out_d[:, 0:4, :], out_sb[:, 0:4, :])
                nc.gpsimd.dma_start(out_d[:, 4:8, :], out_sb[:, 4:8, :])
    _legalize_waits(nc)
    return nc


_nc_cache = None


def _get_nc():
    global _nc_cache
    if _nc_cache is None:
        _nc_cache = _build()
    return _nc_cache


def _marshal(q, k, v, Wq, bq, Wk, bk, Wv, bv):
    """Host-side layout prep: transpose to [B, d_model, N], cast to bf16,
    shard over (batch, query-half)."""
    qT = np.ascontiguousarray(np.transpose(np.asarray(q), (0, 2, 1))).astype(BF)
    kT = np.ascontiguousarray(np.transpose(np.asarray(k), (0, 2, 1))).astype(BF)
    vT = np.ascontiguousarray(np.transpose(np.asarray(v), (0, 2, 1))).astype(BF)
    w3 = np.concatenate(
        [np.asarray(Wq), np.asarray(Wk), np.asarray(Wv)], axis=1
    ).astype(BF)
    # [1024, 192] -> [128, 8*192] partition-major so the DMA is contiguous
    w3 = np.ascontiguousarray(
        w3.reshape(NDM, P, 3 * DK).transpose(1, 0, 2).reshape(P, NDM * 3 * DK)
    )
    b3 = np.stack(
        [np.asarray(bq), np.asarray(bk), np.asarray(bv)], axis=1
    ).astype(np.float32)
    in_maps = []
    for c in range(NCORES):
        bi, h = divmod(c, 2)
        in_maps.append({
            "qT": np.ascontiguousarray(qT[bi][:, h * NQ:(h + 1) * NQ]),
            "kT": kT[bi],
            "vT": vT[bi],
            "w3": w3, "b3": b3,
        })
    return in_maps


def _unmarshal(results, bv):
    out = np.empty((B, N, DK), np.float32)
    for c in range(NCORES):
        bi, h = divmod(c, 2)
        aug = np.transpose(results[c]["out"], (1, 0, 2)).reshape(NQ, DK + 1)
        out[bi, h * NQ:(h + 1) * NQ] = (
            aug[:, :DK] / aug[:, DK:DK + 1] + np.asarray(bv)[None, :]
        )
    return out


def kernel(q, k, v, Wq, bq, Wk, bk, Wv, bv):
    in_maps = _marshal(q, k, v, Wq, bq, Wk, bk, Wv, bv)
    res = run_bass_kernel_spmd(_get_nc(), in_maps, core_ids=list(range(NCORES)))
    return _unmarshal(res.results, bv)


# revision 33
# speedup vs baseline: 1.5608x; 1.0319x over previous
"""Trainium2 Bass kernel for nn_AttentionHead (B=4, N=2048, d_model=1024, d_k=64).

Sharding: data-parallel over (batch, query-half) -> 8 cores. Each core gets
qT[b, :, h*1024:(h+1)*1024], full kT[b], vT[b] (host pre-transposes so d_model
lands on SBUF partitions), plus packed projection weights. Matmuls are bf16
with fp32 PSUM accumulation.

Per-core device program (ACT is the critical engine: 16 exp tiles):
  1. Chunk-major DMA granules of qT/kT stream on the SP/Pool/ACT queues in
     arrival-priority order (q first - scores need all of q_^T but only one
     key tile). ACT preloads the exp table right after its two granules so
     the exp stream starts the moment scores tile 0 lands.
  2. q_^T[64,1024] / k_^T[64,2048] projections accumulate per 512-chunk in
     two rotating PSUM banks (pj0/pj1); DVE writebacks fold the biases.
     Only q and k-chunk-0 gate the loop; k chunks 1-3 and the v chain are
     interleaved into the PE stream at slots where their granules and banks
     are ready.
  3. Main loop over 16 key tiles: scores^T tile [128,1024] in PSUM (2x2
     banks), one Exp ACTIVATE per tile (scale=1/sqrt(dk) folded) into a
     persistent e buffer. v chunks project non-transposed ([keys,64], no
     transposes) straight into v_aug[keys,16,65] whose column 64 is ones.
  4. Out-matmuls are query-major: oacc[qi][128q, 65] += e_t[:,qi]^T @ v_aug_t
     - only a 65-col moving dim on the PE; 8 accumulation groups packed into
     two PSUM banks with one start/stop per bank (zero-region semantics).
     Row 64 accumulates the softmax denominator via the ones column.
  5. No on-device normalize: oacc is written back and DMA'd out as [q,65];
     the host divides by column 64 and adds the v-bias (attn rows sum to 1).

A small legalization pass hoists excess per-instruction semaphore waits onto
same-engine NoOps (this container's walrus accepts at most one).
"""

import numpy as np
import ml_dtypes

import concourse.bass as bass
import concourse.tile as tile
from concourse import mybir
from concourse.bass_utils import run_bass_kernel_spmd

B, N, DM, DK = 4, 2048, 1024, 64
NCORES = 8
NQ = N // 2          # queries per core
NK = N               # keys per core
P = 128
NDM = DM // P        # 8 d_model tiles
NKT = NK // P        # 16 key tiles
NQT = NQ // P        # 8 query tiles
DT = mybir.dt.bfloat16
F32 = mybir.dt.float32
BF = ml_dtypes.bfloat16

ODELAY = 4           # out-matmul lag (tiles) behind the exp stream


# --- walrus wait legalization -------------------------------------------------
# The walrus build in this container accepts at most 1 sync wait + 1 sync
# update per instruction (2 for EventSemaphore). Excess WAITS are hoisted
# onto same-engine NoOps placed just before (queues issue in order, so the
# gating is preserved). Updates are completion-signals and stay put.

def _caps(inst):
    opcode = type(inst).__name__
    if opcode == "InstEventSemaphore":
        return 2, 2
    return 1, 1


def _legalize_waits(nc):
    for f in nc.m.functions:
        for bb in f.blocks:
            out = []
            changed = False
            for inst in bb.instructions:
                si = inst.sync_info
                waits = list(si.on_wait) if si is not None else []
                updates = list(si.on_update) if si is not None else []
                wcap, ucap = _caps(inst)
                if len(waits) <= wcap and len(updates) <= ucap:
                    out.append(inst)
                    continue
                changed = True
                keep_w = waits[len(waits) - wcap:] if wcap else []
                extra_w = waits[: len(waits) - wcap] if wcap else waits
                # Updates signal instruction COMPLETION (writes landed);
                # a following NoOp fires at issue time instead, which races
                # consumers against in-flight writes. Never hoist them.
                assert len(updates) <= ucap, (
                    f"{inst.name}: {len(updates)} sync updates exceed the "
                    f"per-instruction cap and cannot be hoisted safely"
                )
                for w in extra_w:
                    nop = mybir.InstNoOp(
                        name=nc.get_next_instruction_name(), ins=[], outs=[]
                    )
                    nop.engine = inst.engine
                    nop.sync_info = mybir.SyncInfo(on_wait=[w], on_update=[])
                    out.append(nop)
                inst.sync_info = mybir.SyncInfo(on_wait=keep_w, on_update=updates)
                out.append(inst)
            if changed:
                bb.instructions = out


# --- device program -----------------------------------------------------------

def _build(reps=1):
    nc = bass.Bass()
    qT_d = nc.dram_tensor("qT", [DM, NQ], DT, kind="ExternalInput")
    kT_d = nc.dram_tensor("kT", [DM, NK], DT, kind="ExternalInput")
    vT_d = nc.dram_tensor("vT", [DM, NK], DT, kind="ExternalInput")
    w3_d = nc.dram_tensor("w3", [P, NDM * 3 * DK], DT, kind="ExternalInput")
    b3_d = nc.dram_tensor("b3", [DK, 3], F32, kind="ExternalInput")
    out_d = nc.dram_tensor("out", [P, NQT, DK + 1], F32, kind="ExternalOutput")

    EXP = mybir.ActivationFunctionType.Exp
    SCALE = 1.0 / float(np.sqrt(np.float32(DK)))
    QGR = 256            # qT granule width
    VGR = 512            # vT granule width

    with tile.TileContext(nc) as tc:
      for _rep in range(reps):
        with tc.tile_pool(name="persist", bufs=1) as persist:
            w3_sb = persist.tile([P, NDM, 3 * DK], DT, tag="w3_sb")
            b3_sb = persist.tile([DK, 3], F32, tag="b3_sb")
            k_sbT = persist.tile([P, NK], DT, tag="k_sbT")
            q_sbT = persist.tile([P, NQ], DT, tag="q_sbT")
            v_aug = persist.tile([P, NKT, DK + 1], DT, tag="v_aug")
            e_all = persist.tile([P, NKT, NQ], DT, tag="e_all")
            out_sb = persist.tile([P, NQT, DK + 1], F32, tag="out_sb")
            ident = persist.tile([P, P], F32, tag="ident")
            k_nt = persist.tile([P, 4, DK], F32, tag="k_nt")

            with (
                tc.tile_pool(name="xt", bufs=1) as xtp,
                tc.tile_pool(name="pspj", bufs=1, space="PSUM") as pspj,
                tc.tile_pool(name="psscore", bufs=2, space="PSUM") as pss,
                tc.tile_pool(name="psout", bufs=1, space="PSUM") as pso,
            ):
                # ---- DMA streams -------------------------------------------
                # k granule 0a = key tile 0 alone (2KB, lands ~2.6us on ACT)
                # so scores tile 0 unblocks as soon as q_^T completes.
                # SP:   w3, q0, k0b, k2, v0, v2      (+ out half 0)
                # Pool: q1, q3, k1, k3, v1, v3      (+ out half 1)
                # ACT:  b3, k0a, q2, exp table, then the exp stream.
                kgr = {}
                qgr, vgr = [None] * 4, [None] * 4

                def dma_k(nm, lo, hi, eng):
                    t_ = xtp.tile([P, NDM, hi - lo], DT, tag=f"kg{nm}",
                                  name=f"kg{nm}")
                    eng.dma_start(
                        t_[:], kT_d[:, lo:hi].rearrange(
                            "(o p) n -> p o n", p=P))
                    kgr[nm] = t_

                def dma_q(i, eng):
                    t_ = xtp.tile([P, NDM, QGR], DT, tag=f"qg{i}", name=f"qg{i}")
                    eng.dma_start(
                        t_[:], qT_d[:, i * QGR:(i + 1) * QGR].rearrange(
                            "(o p) n -> p o n", p=P))
                    qgr[i] = t_

                def dma_v(i, eng):
                    t_ = xtp.tile([P, NDM, VGR], DT, tag=f"vg{i}", name=f"vg{i}")
                    eng.dma_start(
                        t_[:], vT_d[:, i * VGR:(i + 1) * VGR].rearrange(
                            "(o p) n -> p o n", p=P))
                    vgr[i] = t_

                nc.sync.dma_start(
                    w3_sb[:], w3_d.rearrange("p (o k) -> p o k", o=NDM))
                nc.scalar.dma_start(b3_sb[:], b3_d[:])
                dma_q(1, nc.gpsimd)
                dma_k("0a", 0, 256, nc.scalar)
                dma_q(0, nc.sync)
                dma_q(3, nc.gpsimd)
                dma_q(2, nc.scalar)
                dma_k("0b", 256, 512, nc.sync)
                # identity for the k-chunk transposes rides the Pool queue
                # between granules (affine_select is gpsimd-only)
                from concourse.masks import make_identity
                make_identity(nc, ident[:])
                dma_k("1", 512, 1024, nc.gpsimd)
                dma_v(0, nc.sync)
                dma_k("3", 1536, 2048, nc.gpsimd)
                dma_k("2", 1024, 1536, nc.sync)
                dma_v(1, nc.gpsimd)
                dma_v(2, nc.sync)
                dma_v(3, nc.gpsimd)

                # preload the exp table while the DMA streams run
                nc.scalar.activation(
                    e_all[0:1, 0, 0:1], b3_sb[0:1, 0:1], EXP, scale=1.0)
                # zero-pad dead partitions (scores contract over 128; NaN
                # poison in uninitialized SBUF would survive 0*NaN)
                nc.vector.memset(q_sbT[DK:P, :], 0.0)
                nc.vector.memset(k_sbT[DK:P, :], 0.0)
                # ones column -> row 64 of out accumulates the denominator
                nc.vector.memset(v_aug[:, :, DK:DK + 1], 1.0)

                # ---- rotating projection / v-chain PSUM banks --------------
                def pjtile(which, name):
                    return pspj.tile([P, 512], F32, tag=f"pj{which}", name=name)

                oacc = [
                    pso.tile([P, 4, DK + 1], F32, tag=f"oacc{g}",
                             name=f"oacc{g}")
                    for g in range(2)
                ]

                def proj(ps, gr, off, wid, wlo, start, stop, glo=0):
                    # accumulate granule cols [glo, glo+wid) into psum bank
                    # cols [off, off+wid)
                    for dmt in range(NDM):
                        nc.tensor.matmul(
                            ps[0:DK, off:off + wid],
                            w3_sb[:, dmt, wlo:wlo + DK],
                            gr[:, dmt, glo:glo + wid],
                            start=(start and dmt == 0),
                            stop=(stop and dmt == NDM - 1))

                def wb_k(ps, off, wid, dst):
                    nc.vector.tensor_scalar_add(
                        k_sbT[0:DK, dst:dst + wid], ps[0:DK, off:off + wid],
                        b3_sb[:, 1:2])

                def wb_q(ps, off, wid, dst):
                    nc.vector.tensor_scalar_add(
                        q_sbT[0:DK, dst:dst + wid], ps[0:DK, off:off + wid],
                        b3_sb[:, 0:1])

                def v_mm(pv, j, klo, khi, start, stop):
                    # project v chunk-j key tiles [klo,khi) (global indices,
                    # non-transposed); tile kt sits at pv cols (kt%4)*128
                    for dmt in range(NDM):
                        for kt in range(klo, khi):
                            lo = (kt - 4 * j) * P
                            nc.tensor.matmul(
                                pv[:, lo:lo + DK],
                                vgr[j][:, dmt, lo:lo + P],
                                w3_sb[:, dmt, 2 * DK:3 * DK],
                                start=(start and dmt == 0 and kt == klo),
                                stop=(stop and dmt == NDM - 1
                                      and kt == khi - 1))

                def v_wb(pv, j, klo, khi):
                    src = pv[:].rearrange("p (k c) -> p k c", c=P)
                    nc.vector.tensor_copy(
                        v_aug[:, klo:khi, 0:DK],
                        src[:, klo - 4 * j:khi - 4 * j, 0:DK])

                def o_mm(t):
                    for qi in range(NQT):
                        nc.tensor.matmul(
                            oacc[qi // 4][:, qi % 4, :],
                            e_all[:, t, qi * P:(qi + 1) * P],
                            v_aug[:, t, :],
                            start=(t == 0 and qi % 4 == 0),
                            stop=(t == NKT - 1 and qi % 4 == 3))

                def emit_scores(t):
                    sc = pss.tile([P, NQ], F32, tag="psscore", name=f"sc{t}")
                    for h in range(2):
                        nc.tensor.matmul(
                            sc[:, h * 512:(h + 1) * 512],
                            k_sbT[:, t * P:(t + 1) * P],
                            q_sbT[:, h * 512:(h + 1) * 512],
                            start=True, stop=True)
                    return sc

                # ---- pre-loop: q projection + k tile 0 ---------------------
                # emission order tracks expected granule arrival; per-granule
                # writebacks keep the critical chains short. k tiles 0-3
                # accumulate in the first scores buffer (idle until sc1).
                psk0 = pss.tile([P, NQ], F32, tag="psscore", name="psk0")
                psq0 = pjtile(0, "psq0")
                psq1 = pjtile(1, "psq1")
                proj(psq0, qgr[1], 256, 256, 0, start=True, stop=False)
                proj(psq0, qgr[0], 0, 256, 0, start=False, stop=True)
                wb_q(psq0, 0, 512, 0)
                proj(psk0, kgr["0a"], 0, 256, DK, start=True, stop=False)
                wb_k(psk0, 0, 256, 0)
                proj(psq1, qgr[2], 0, 256, 0, start=True, stop=False)
                proj(psq1, qgr[3], 256, 256, 0, start=False, stop=True)
                wb_q(psq1, 0, 512, 512)

                # ---- main loop: scores -> exp -> (k/v chains, out-mms) -----
                # per-slot extra PE work, keyed by loop slot index
                kwork = {}   # slot -> list of thunks

                def add(slot, fn):
                    kwork.setdefault(slot, []).append(fn)

                def mk(fn):          # bind loop vars eagerly
                    return fn

                # k chunks 1-3: non-transposed projection (64-row matmuls,
                # 2.5x cheaper on the PE) + 4 PE transposes per chunk; the
                # bias folds into the transpose writeback
                psk_t = [None]
                def knt_mm(nm, c):
                    psk_t[0] = pjtile(1, f"pkn{c}")
                    ps = psk_t[0]
                    for dmt in range(NDM):
                        for i in range(4):
                            nc.tensor.matmul(
                                ps[:, i * DK:(i + 1) * DK],
                                kgr[nm][:, dmt, i * P:(i + 1) * P],
                                w3_sb[:, dmt, DK:2 * DK],
                                start=(dmt == 0 and i == 0),
                                stop=(dmt == NDM - 1 and i == 3))
                    nc.vector.tensor_copy(
                        k_nt[:], ps[:, 0:4 * DK].rearrange(
                            "p (k c) -> p k c", c=DK))

                def knt_tr(c):
                    ps = psk_t[0]
                    for i in range(4):
                        nc.tensor.matmul(
                            ps[0:DK, i * P:(i + 1) * P], k_nt[:, i, :],
                            ident[:], is_transpose=True,
                            start=(i == 0), stop=(i == 3))
                    nc.vector.tensor_scalar_add(
                        k_sbT[0:DK, c * 512:(c + 1) * 512], ps[0:DK, :],
                        b3_sb[:, 1:2])

                psv_t = [None]
                def psv_mm(j, klo, khi, start, stop):
                    if start:
                        psv_t[0] = pjtile(0, f"psv{j}_{klo}")
                    pv = psv_t[0]
                    v_mm(pv, j, klo, khi, start, stop)
                    v_wb(pv, j, klo, khi)

                # k chunk c: projection matmuls at slot s0, transposes (which
                # wait on the chunk writeback round-trip) one slot later
                for c, nm, s0 in ((1, "1", 0), (2, "2", 4), (3, "3", 6)):
                    add(s0, mk(lambda nm=nm, c=c: knt_mm(nm, c)))
                    add(s0 + 1, mk(lambda c=c: knt_tr(c)))
                # v sub-chains, two key tiles at a time
                add(2, mk(lambda: psv_mm(0, 0, 2, True, False)))
                add(5, mk(lambda: psv_mm(0, 2, 4, False, True)))
                add(8, mk(lambda: psv_mm(1, 4, 6, True, False)))
                add(9, mk(lambda: psv_mm(1, 6, 8, False, True)))
                add(10, mk(lambda: psv_mm(2, 8, 10, True, False)))
                add(11, mk(lambda: psv_mm(2, 10, 12, False, True)))
                add(12, mk(lambda: psv_mm(3, 12, 14, True, False)))
                add(13, mk(lambda: psv_mm(3, 14, 16, False, True)))

                sc_cur = emit_scores(0)
                # k chunk 0 tiles 2-3 (granule k0b) finish in psk0; this must
                # precede sc1, which rotates back onto psk0's buffer
                proj(psk0, kgr["0b"], 256, 256, DK, start=False, stop=True)
                wb_k(psk0, 256, 256, 256)
                for t in range(NKT + ODELAY):
                    if t < NKT:
                        if t + 1 < NKT:
                            sc_next = emit_scores(t + 1)
                        nc.scalar.activation(
                            e_all[:, t, :], sc_cur[:], EXP, scale=SCALE)
                        if t + 1 < NKT:
                            sc_cur = sc_next
                    for fn in kwork.get(t, []):
                        fn()
                    if t >= ODELAY:
                        o_mm(t - ODELAY)

                # writeback + store (host does the softmax divide); the two
                # halves write back on DVE and ACT in parallel, and the
                # partition-major out layout gives one 1KB descriptor per
                # partition per DMA
                nc.vector.tensor_copy(out_sb[:, 0:4, :], oacc[0][:])
                nc.scalar.copy(out_sb[:, 4:8, :], oacc[1][:])
                nc.sync.dma_start(out_d[:, 0:4, :], out_sb[:, 0:4, :])
                nc.gpsimd.dma_start(out_d[:, 4:8, :], out_sb[:, 4:8, :])
    _legalize_waits(nc)
    return nc


_nc_cache = None


def _get_nc():
    global _nc_cache
    if _nc_cache is None:
        _nc_cache = _build()
    return _nc_cache


def _marshal(q, k, v, Wq, bq, Wk, bk, Wv, bv):
    """Host-side layout prep: transpose to [B, d_model, N], cast to bf16,
    shard over (batch, query-half)."""
    qT = np.ascontiguousarray(np.transpose(np.asarray(q), (0, 2, 1))).astype(BF)
    kT = np.ascontiguousarray(np.transpose(np.asarray(k), (0, 2, 1))).astype(BF)
    vT = np.ascontiguousarray(np.transpose(np.asarray(v), (0, 2, 1))).astype(BF)
    w3 = np.concatenate(
        [np.asarray(Wq), np.asarray(Wk), np.asarray(Wv)], axis=1
    ).astype(BF)
    # [1024, 192] -> [128, 8*192] partition-major so the DMA is contiguous
    w3 = np.ascontiguousarray(
        w3.reshape(NDM, P, 3 * DK).transpose(1, 0, 2).reshape(P, NDM * 3 * DK)
    )
    b3 = np.stack(
        [np.asarray(bq), np.asarray(bk), np.asarray(bv)], axis=1
    ).astype(np.float32)
    in_maps = []
    for c in range(NCORES):
        bi, h = divmod(c, 2)
        in_maps.append({
            "qT": np.ascontiguousarray(qT[bi][:, h * NQ:(h + 1) * NQ]),
            "kT": kT[bi],
            "vT": vT[bi],
            "w3": w3, "b3": b3,
        })
    return in_maps


def _unmarshal(results, bv):
    out = np.empty((B, N, DK), np.float32)
    for c in range(NCORES):
        bi, h = divmod(c, 2)
        aug = np.transpose(results[c]["out"], (1, 0, 2)).reshape(NQ, DK + 1)
        out[bi, h * NQ:(h + 1) * NQ] = (
            aug[:, :DK] / aug[:, DK:DK + 1] + np.asarray(bv)[None, :]
        )
    return out


def kernel(q, k, v, Wq, bq, Wk, bk, Wv, bv):
    in_maps = _marshal(q, k, v, Wq, bq, Wk, bk, Wv, bv)
    res = run_bass_kernel_spmd(_get_nc(), in_maps, core_ids=list(range(NCORES)))
    return _unmarshal(res.results, bv)


# revision 36
# speedup vs baseline: 1.5748x; 1.0090x over previous
"""Trainium2 Bass kernel for nn_AttentionHead (B=4, N=2048, d_model=1024, d_k=64).

Sharding: data-parallel over (batch, query-half) -> 8 cores. Each core gets
qT[b, :, h*1024:(h+1)*1024], full kT[b], vT[b] (host pre-transposes so d_model
lands on SBUF partitions), plus packed projection weights. Matmuls are bf16
with fp32 PSUM accumulation.

Per-core device program (ACT is the critical engine: 16 exp tiles):
  1. Chunk-major DMA granules of qT/kT stream on the SP/Pool/ACT queues in
     arrival-priority order (q first - scores need all of q_^T but only one
     key tile). ACT preloads the exp table right after its two granules so
     the exp stream starts the moment scores tile 0 lands.
  2. q_^T[64,1024] / k_^T[64,2048] projections accumulate per 512-chunk in
     two rotating PSUM banks (pj0/pj1); DVE writebacks fold the biases.
     Only q and k-chunk-0 gate the loop; k chunks 1-3 and the v chain are
     interleaved into the PE stream at slots where their granules and banks
     are ready.
  3. Main loop over 16 key tiles: scores^T tile [128,1024] in PSUM (2x2
     banks), one Exp ACTIVATE per tile (scale=1/sqrt(dk) folded) into a
     persistent e buffer. v chunks project non-transposed ([keys,64], no
     transposes) straight into v_aug[keys,16,65] whose column 64 is ones.
  4. Out-matmuls are query-major: oacc[qi][128q, 65] += e_t[:,qi]^T @ v_aug_t
     - only a 65-col moving dim on the PE; 8 accumulation groups packed into
     two PSUM banks with one start/stop per bank (zero-region semantics).
     Row 64 accumulates the softmax denominator via the ones column.
  5. No on-device normalize: oacc is written back and DMA'd out as [q,65];
     the host divides by column 64 and adds the v-bias (attn rows sum to 1).

A small legalization pass hoists excess per-instruction semaphore waits onto
same-engine NoOps (this container's walrus accepts at most one).
"""

import numpy as np
import ml_dtypes

import concourse.bass as bass
import concourse.tile as tile
from concourse import mybir
from concourse.bass_utils import run_bass_kernel_spmd

B, N, DM, DK = 4, 2048, 1024, 64
NCORES = 8
NQ = N // 2          # queries per core
NK = N               # keys per core
P = 128
NDM = DM // P        # 8 d_model tiles
NKT = NK // P        # 16 key tiles
NQT = NQ // P        # 8 query tiles
DT = mybir.dt.bfloat16
F32 = mybir.dt.float32
BF = ml_dtypes.bfloat16

ODELAY = 4           # out-matmul lag (tiles) behind the exp stream


# --- walrus wait legalization -------------------------------------------------
# The walrus build in this container accepts at most 1 sync wait + 1 sync
# update per instruction (2 for EventSemaphore). Excess WAITS are hoisted
# onto same-engine NoOps placed just before (queues issue in order, so the
# gating is preserved). Updates are completion-signals and stay put.

def _caps(inst):
    opcode = type(inst).__name__
    if opcode == "InstEventSemaphore":
        return 2, 2
    return 1, 1


def _legalize_waits(nc):
    for f in nc.m.functions:
        for bb in f.blocks:
            out = []
            changed = False
            for inst in bb.instructions:
                si = inst.sync_info
                waits = list(si.on_wait) if si is not None else []
                updates = list(si.on_update) if si is not None else []
                wcap, ucap = _caps(inst)
                if len(waits) <= wcap and len(updates) <= ucap:
                    out.append(inst)
                    continue
                changed = True
                keep_w = waits[len(waits) - wcap:] if wcap else []
                extra_w = waits[: len(waits) - wcap] if wcap else waits
                # Updates signal instruction COMPLETION (writes landed);
                # a following NoOp fires at issue time instead, which races
                # consumers against in-flight writes. Never hoist them.
                assert len(updates) <= ucap, (
                    f"{inst.name}: {len(updates)} sync updates exceed the "
                    f"per-instruction cap and cannot be hoisted safely"
                )
                for w in extra_w:
                    nop = mybir.InstNoOp(
                        name=nc.get_next_instruction_name(), ins=[], outs=[]
                    )
                    nop.engine = inst.engine
                    nop.sync_info = mybir.SyncInfo(on_wait=[w], on_update=[])
                    out.append(nop)
                inst.sync_info = mybir.SyncInfo(on_wait=keep_w, on_update=updates)
                out.append(inst)
            if changed:
                bb.instructions = out


# --- device program -----------------------------------------------------------

def _build(reps=1):
    nc = bass.Bass()
    qT_d = nc.dram_tensor("qT", [DM, NQ], DT, kind="ExternalInput")
    kT_d = nc.dram_tensor("kT", [DM, NK], DT, kind="ExternalInput")
    vT_d = nc.dram_tensor("vT", [DM, NK], DT, kind="ExternalInput")
    w3_d = nc.dram_tensor("w3", [P, NDM * 3 * DK], DT, kind="ExternalInput")
    b3_d = nc.dram_tensor("b3", [DK, 3], F32, kind="ExternalInput")
    out_d = nc.dram_tensor("out", [P, NQT, DK + 1], F32, kind="ExternalOutput")

    EXP = mybir.ActivationFunctionType.Exp
    SCALE = 1.0 / float(np.sqrt(np.float32(DK)))
    QGR = 256            # qT granule width
    VGR = 512            # vT granule width

    with tile.TileContext(nc) as tc:
      for _rep in range(reps):
        with tc.tile_pool(name="persist", bufs=1) as persist:
            w3_sb = persist.tile([P, NDM, 3 * DK], DT, tag="w3_sb")
            b3_sb = persist.tile([DK, 3], F32, tag="b3_sb")
            k_sbT = persist.tile([P, NK], DT, tag="k_sbT")
            q_sbT = persist.tile([P, NQ], DT, tag="q_sbT")
            v_aug = persist.tile([P, NKT, DK + 1], DT, tag="v_aug")
            e_all = persist.tile([P, NKT, NQ], DT, tag="e_all")
            out_sb = persist.tile([P, NQT, DK + 1], F32, tag="out_sb")
            ident = persist.tile([P, P], F32, tag="ident")
            k_nt = persist.tile([P, 4, DK], F32, tag="k_nt")

            with (
                tc.tile_pool(name="xt", bufs=1) as xtp,
                tc.tile_pool(name="pspj", bufs=1, space="PSUM") as pspj,
                tc.tile_pool(name="psscore", bufs=2, space="PSUM") as pss,
                tc.tile_pool(name="psout", bufs=1, space="PSUM") as pso,
            ):
                # ---- DMA streams -------------------------------------------
                # k granule 0a = key tile 0 alone (2KB, lands ~2.6us on ACT)
                # so scores tile 0 unblocks as soon as q_^T completes.
                # SP:   w3, q0, k0b, k2, v0, v2      (+ out half 0)
                # Pool: q1, q3, k1, k3, v1, v3      (+ out half 1)
                # ACT:  b3, k0a, q2, exp table, then the exp stream.
                kgr = {}
                qgr, vgr = [None] * 4, [None] * 4

                def dma_k(nm, lo, hi, eng):
                    t_ = xtp.tile([P, NDM, hi - lo], DT, tag=f"kg{nm}",
                                  name=f"kg{nm}")
                    eng.dma_start(
                        t_[:], kT_d[:, lo:hi].rearrange(
                            "(o p) n -> p o n", p=P))
                    kgr[nm] = t_

                def dma_q(i, eng):
                    t_ = xtp.tile([P, NDM, QGR], DT, tag=f"qg{i}", name=f"qg{i}")
                    eng.dma_start(
                        t_[:], qT_d[:, i * QGR:(i + 1) * QGR].rearrange(
                            "(o p) n -> p o n", p=P))
                    qgr[i] = t_

                def dma_v(i, eng):
                    t_ = xtp.tile([P, NDM, VGR], DT, tag=f"vg{i}", name=f"vg{i}")
                    eng.dma_start(
                        t_[:], vT_d[:, i * VGR:(i + 1) * VGR].rearrange(
                            "(o p) n -> p o n", p=P))
                    vgr[i] = t_

                nc.sync.dma_start(
                    w3_sb[:], w3_d.rearrange("p (o k) -> p o k", o=NDM))
                nc.scalar.dma_start(b3_sb[:], b3_d[:])
                dma_q(1, nc.gpsimd)
                dma_k("0a", 0, 256, nc.scalar)
                dma_q(0, nc.sync)
                dma_q(3, nc.gpsimd)
                dma_q(2, nc.scalar)
                dma_k("0b", 256, 512, nc.sync)
                # identity for the k-chunk transposes rides the Pool queue
                # between granules (affine_select is gpsimd-only)
                from concourse.masks import make_identity
                make_identity(nc, ident[:])
                dma_k("1", 512, 1024, nc.gpsimd)
                dma_v(0, nc.sync)
                dma_k("3", 1536, 2048, nc.gpsimd)
                dma_k("2", 1024, 1536, nc.sync)
                dma_v(1, nc.gpsimd)
                dma_v(2, nc.sync)
                dma_v(3, nc.gpsimd)

                # preload the exp table while the DMA streams run
                nc.scalar.activation(
                    e_all[0:1, 0, 0:1], b3_sb[0:1, 0:1], EXP, scale=1.0)
                # zero-pad dead partitions (scores contract over 128; NaN
                # poison in uninitialized SBUF would survive 0*NaN)
                nc.vector.memset(q_sbT[DK:P, :], 0.0)
                nc.vector.memset(k_sbT[DK:P, :], 0.0)
                # ones column -> row 64 of out accumulates the denominator
                nc.vector.memset(v_aug[:, :, DK:DK + 1], 1.0)

                # ---- rotating projection / v-chain PSUM banks --------------
                def pjtile(which, name):
                    return pspj.tile([P, 512], F32, tag=f"pj{which}", name=name)

                oacc = [
                    pso.tile([P, 4, DK + 1], F32, tag=f"oacc{g}",
                             name=f"oacc{g}")
                    for g in range(2)
                ]

                def proj(ps, gr, off, wid, wlo, start, stop, glo=0):
                    # accumulate granule cols [glo, glo+wid) into psum bank
                    # cols [off, off+wid)
                    for dmt in range(NDM):
                        nc.tensor.matmul(
                            ps[0:DK, off:off + wid],
                            w3_sb[:, dmt, wlo:wlo + DK],
                            gr[:, dmt, glo:glo + wid],
                            start=(start and dmt == 0),
                            stop=(stop and dmt == NDM - 1))

                def wb_k(ps, off, wid, dst):
                    nc.vector.tensor_scalar_add(
                        k_sbT[0:DK, dst:dst + wid], ps[0:DK, off:off + wid],
                        b3_sb[:, 1:2])

                def wb_q(ps, off, wid, dst):
                    nc.vector.tensor_scalar_add(
                        q_sbT[0:DK, dst:dst + wid], ps[0:DK, off:off + wid],
                        b3_sb[:, 0:1])

                def v_mm(pv, j, klo, khi, start, stop):
                    # project v chunk-j key tiles [klo,khi) (global indices,
                    # non-transposed); tile kt sits at pv cols (kt%4)*128
                    for dmt in range(NDM):
                        for kt in range(klo, khi):
                            lo = (kt - 4 * j) * P
                            nc.tensor.matmul(
                                pv[:, lo:lo + DK],
                                vgr[j][:, dmt, lo:lo + P],
                                w3_sb[:, dmt, 2 * DK:3 * DK],
                                start=(start and dmt == 0 and kt == klo),
                                stop=(stop and dmt == NDM - 1
                                      and kt == khi - 1))

                def v_wb(pv, j, klo, khi):
                    src = pv[:].rearrange("p (k c) -> p k c", c=P)
                    nc.vector.tensor_copy(
                        v_aug[:, klo:khi, 0:DK],
                        src[:, klo - 4 * j:khi - 4 * j, 0:DK])

                def o_mm(t):
                    for qi in range(NQT):
                        nc.tensor.matmul(
                            oacc[qi // 4][:, qi % 4, :],
                            e_all[:, t, qi * P:(qi + 1) * P],
                            v_aug[:, t, :],
                            start=(t == 0 and qi % 4 == 0),
                            stop=(t == NKT - 1 and qi % 4 == 3))

                def emit_scores(t):
                    sc = pss.tile([P, NQ], F32, tag="psscore", name=f"sc{t}")
                    for h in range(2):
                        nc.tensor.matmul(
                            sc[:, h * 512:(h + 1) * 512],
                            k_sbT[:, t * P:(t + 1) * P],
                            q_sbT[:, h * 512:(h + 1) * 512],
                            start=True, stop=True)
                    return sc

                # ---- pre-loop: q projection + k tile 0 ---------------------
                # emission order tracks expected granule arrival; per-granule
                # writebacks keep the critical chains short. k tiles 0-3
                # accumulate in the first scores buffer (idle until sc1).
                psk0 = pss.tile([P, NQ], F32, tag="psscore", name="psk0")
                psq0 = pjtile(0, "psq0")
                psq1 = pjtile(1, "psq1")
                proj(psq0, qgr[1], 256, 256, 0, start=True, stop=False)
                proj(psq0, qgr[0], 0, 256, 0, start=False, stop=True)
                wb_q(psq0, 0, 512, 0)
                proj(psk0, kgr["0a"], 0, 256, DK, start=True, stop=False)
                wb_k(psk0, 0, 256, 0)
                proj(psq1, qgr[2], 0, 256, 0, start=True, stop=False)
                proj(psq1, qgr[3], 256, 256, 0, start=False, stop=True)
                wb_q(psq1, 0, 512, 512)

                # ---- main loop: scores -> exp -> (k/v chains, out-mms) -----
                # per-slot extra PE work, keyed by loop slot index
                kwork = {}   # slot -> list of thunks

                def add(slot, fn):
                    kwork.setdefault(slot, []).append(fn)

                def mk(fn):          # bind loop vars eagerly
                    return fn

                # k chunks 1-3: non-transposed projection (64-row matmuls,
                # 2.5x cheaper on the PE) + 4 PE transposes per chunk; the
                # bias folds into the transpose writeback
                psk_t = [None]
                def knt_mm(nm, c):
                    psk_t[0] = pjtile(1, f"pkn{c}")
                    ps = psk_t[0]
                    for dmt in range(NDM):
                        for i in range(4):
                            nc.tensor.matmul(
                                ps[:, i * DK:(i + 1) * DK],
                                kgr[nm][:, dmt, i * P:(i + 1) * P],
                                w3_sb[:, dmt, DK:2 * DK],
                                start=(dmt == 0 and i == 0),
                                stop=(dmt == NDM - 1 and i == 3))
                    nc.vector.tensor_copy(
                        k_nt[:], ps[:, 0:4 * DK].rearrange(
                            "p (k c) -> p k c", c=DK))

                def knt_tr(c):
                    ps = psk_t[0]
                    for i in range(4):
                        nc.tensor.matmul(
                            ps[0:DK, i * P:(i + 1) * P], k_nt[:, i, :],
                            ident[:], is_transpose=True,
                            start=(i == 0), stop=(i == 3))
                    nc.vector.tensor_scalar_add(
                        k_sbT[0:DK, c * 512:(c + 1) * 512], ps[0:DK, :],
                        b3_sb[:, 1:2])

                psv_t = [None]
                def psv_mm(j, klo, khi, start, stop):
                    if start:
                        psv_t[0] = pjtile(0, f"psv{j}_{klo}")
                    pv = psv_t[0]
                    v_mm(pv, j, klo, khi, start, stop)
                    v_wb(pv, j, klo, khi)

                # k chunk c: projection matmuls at slot s0, transposes (which
                # wait on the chunk writeback round-trip) one slot later
                for c, nm, s0 in ((1, "1", 0), (2, "2", 3), (3, "3", 6)):
                    add(s0, mk(lambda nm=nm, c=c: knt_mm(nm, c)))
                    add(s0 + 1, mk(lambda c=c: knt_tr(c)))
                # v sub-chains, two key tiles at a time
                add(2, mk(lambda: psv_mm(0, 0, 2, True, False)))
                add(5, mk(lambda: psv_mm(0, 2, 4, False, True)))
                add(8, mk(lambda: psv_mm(1, 4, 6, True, False)))
                add(9, mk(lambda: psv_mm(1, 6, 8, False, True)))
                add(10, mk(lambda: psv_mm(2, 8, 10, True, False)))
                add(11, mk(lambda: psv_mm(2, 10, 12, False, True)))
                add(12, mk(lambda: psv_mm(3, 12, 14, True, False)))
                add(13, mk(lambda: psv_mm(3, 14, 16, False, True)))

                sc_cur = emit_scores(0)
                # k chunk 0 tiles 2-3 (granule k0b) finish in psk0; this must
                # precede sc1, which rotates back onto psk0's buffer
                proj(psk0, kgr["0b"], 256, 256, DK, start=False, stop=True)
                wb_k(psk0, 256, 256, 256)
                for t in range(NKT + ODELAY):
                    if t < NKT:
                        if t + 1 < NKT:
                            sc_next = emit_scores(t + 1)
                        nc.scalar.activation(
                            e_all[:, t, :], sc_cur[:], EXP, scale=SCALE)
                        if t + 1 < NKT:
                            sc_cur = sc_next
                    for fn in kwork.get(t, []):
                        fn()
                    if t >= ODELAY:
                        o_mm(t - ODELAY)

                # writeback + store (host does the softmax divide); the two
                # halves write back on DVE and ACT in parallel, and the
                # partition-major out layout gives one 1KB descriptor per
                # partition per DMA
                nc.vector.tensor_copy(out_sb[:, 0:4, :], oacc[0][:])
                nc.scalar.copy(out_sb[:, 4:8, :], oacc[1][:])
                nc.sync.dma_start(out_d[:, 0:4, :], out_sb[:, 0:4, :])
                nc.scalar.dma_start(out_d[:, 4:8, :], out_sb[:, 4:8, :])
    _legalize_waits(nc)
    return nc


_nc_cache = None


def _get_nc():
    global _nc_cache
    if _nc_cache is None:
        _nc_cache = _build()
    return _nc_cache


def _marshal(q, k, v, Wq, bq, Wk, bk, Wv, bv):
    """Host-side layout prep: transpose to [B, d_model, N], cast to bf16,
    shard over (batch, query-half)."""
    qT = np.ascontiguousarray(np.transpose(np.asarray(q), (0, 2, 1))).astype(BF)
    kT = np.ascontiguousarray(np.transpose(np.asarray(k), (0, 2, 1))).astype(BF)
    vT = np.ascontiguousarray(np.transpose(np.asarray(v), (0, 2, 1))).astype(BF)
    w3 = np.concatenate(
        [np.asarray(Wq), np.asarray(Wk), np.asarray(Wv)], axis=1
    ).astype(BF)
    # [1024, 192] -> [128, 8*192] partition-major so the DMA is contiguous
    w3 = np.ascontiguousarray(
        w3.reshape(NDM, P, 3 * DK).transpose(1, 0, 2).reshape(P, NDM * 3 * DK)
    )
    b3 = np.stack(
        [np.asarray(bq), np.asarray(bk), np.asarray(bv)], axis=1
    ).astype(np.float32)
    in_maps = []
    for c in range(NCORES):
        bi, h = divmod(c, 2)
        in_maps.append({
            "qT": np.ascontiguousarray(qT[bi][:, h * NQ:(h + 1) * NQ]),
            "kT": kT[bi],
            "vT": vT[bi],
            "w3": w3, "b3": b3,
        })
    return in_maps


def _unmarshal(results, bv):
    out = np.empty((B, N, DK), np.float32)
    for c in range(NCORES):
        bi, h = divmod(c, 2)
        aug = np.transpose(results[c]["out"], (1, 0, 2)).reshape(NQ, DK + 1)
        out[bi, h * NQ:(h + 1) * NQ] = (
            aug[:, :DK] / aug[:, DK:DK + 1] + np.asarray(bv)[None, :]
        )
    return out


def kernel(q, k, v, Wq, bq, Wk, bk, Wv, bv):
    in_maps = _marshal(q, k, v, Wq, bq, Wk, bk, Wv, bv)
    res = run_bass_kernel_spmd(_get_nc(), in_maps, core_ids=list(range(NCORES)))
    return _unmarshal(res.results, bv)
